# revision 8
# baseline (speedup 1.0000x reference)
"""MLA prefill kernel for 8 trn2 NeuronCores.

Sharding: core c handles batch b = c//4, head group hg = c%4 (4 of 16 heads).
Each core computes its 4 heads' attention + its partial proj output
[T, C]; the host sums the 4 partials per batch (unshard of the
head-contracted proj output) and stacks batches.

Device dataflow per core (matmuls in bf16, psum fp32):
  A: q_nope^T [4x128, T] -> SBUF-resident qnopeSB; q_rope [t,d] rotated
     -> SBUF-resident qrotSB
  B: ckv [t, 576] -> rmsnorm(lat), rotate k_rope, PE-transpose to
     ckv'^T [4x128, T], kropeT(dup) [128, T], qropeTz [4x(128, T)]
     zero-padded per head so phase D rope matmuls contract K=128
  C: k_nope^T per head [128, T], v [t, 4*128] -> SBUF-resident vSB
  D: per (head, q-block 512): S^T tiles [128k, 512q] (nope+rope matmuls),
     exp via ACT from PSUM (masked subblocks via DVE STT), row-sum l via
     DVE accumulation of sprime chunks + one fp32r ones-matmul,
     att^T accum via V-matmul; normalize by exp(-ln l); proj accumulates
     4 head-chunks -> direct PSUM->HBM DMA.
  One PSUM pool with per-bank tags spans all phases (no pool barriers).
"""

import numpy as np

B, T, C, H = 2, 2048, 2048, 16
NOPE, ROPE, VD, LORA = 128, 64, 128, 512
QK = NOPE + ROPE
EPS = 1e-6
SCALE = 1.0 / float(np.sqrt(QK))
P = 128
KC = C // P    # 16 contraction chunks over C
TB = T // P    # 16 token sub-blocks
NQ = T // 512  # 4 q-blocks
HPC = 4        # heads per core
N_CORES = 8

_prog_cache = {}
LAST_RESULTS = None  # BassKernelResults of the most recent run (for test.py)
MM_DTYPE = "bfloat16"  # "bfloat16" or "float32r" for all matmul operands


# ---------------------------------------------------------------- host prep

def _mask_plan(mask):
    """Classify mask into per-(q-block, k-chunk) plans.

    plan[j] = list of (c, col0, subops); subops[qs] in
    {"skip", "zero", ("g", gidx)} for columns [128*qs, 128*qs+128) of the
    S^T tile. col0 = 128 * (# leading skip sub-blocks), forced to 0 for
    the first chunk of each j. Chunks with all sub-blocks skip are
    omitted (their softmax contribution is exactly 0 in fp32).
    """
    plan = []
    gblocks = []
    for j in range(NQ):
        chunks = []
        for c in range(TB):
            sub = []
            nskip_lead = 0
            leading = True
            any_alive = False
            for qs in range(4):
                blk = mask[512 * j + 128 * qs: 512 * j + 128 * qs + 128,
                           128 * c: 128 * c + 128]
                if np.all(blk <= -88.0):
                    sub.append("skip")
                    if leading:
                        nskip_lead += 1
                elif np.all(blk == 0.0):
                    sub.append("zero")
                    leading = False
                    any_alive = True
                else:
                    gidx = len(gblocks)
                    gblocks.append(np.ascontiguousarray(blk.T))
                    sub.append(("g", gidx))
                    leading = False
                    any_alive = True
            if not any_alive:
                continue
            col0 = 128 * nskip_lead
            if not chunks:
                col0 = 0  # first chunk must initialize full psum width
            chunks.append((c, col0, sub))
        assert chunks, f"q-block {j}: all keys masked (unsupported)"
        plan.append(chunks)
    if gblocks:
        garr = np.stack(gblocks).astype(np.float32)
    else:
        garr = np.zeros((1, 128, 128), np.float32)
    return plan, garr


def _plan_key(plan):
    return tuple(
        tuple((c, col0, tuple(s if isinstance(s, str) else ("g",) for s in sub))
              for (c, col0, sub) in chunks)
        for chunks in plan
    )


def _pack_wq(Wq, hg):
    """[C, 768]: 4 heads' nope cols, then 2 rope 'pair' chunks laid out
    [h_even(32) h_odd(32) h'_even(32) h'_odd(32)]."""
    heads = [4 * hg + i for i in range(HPC)]
    cols = [Wq[:, h * QK: h * QK + NOPE] for h in heads]
    for h in heads:
        rope = Wq[:, h * QK + NOPE: h * QK + QK]
        cols.append(rope[:, 0::2])
        cols.append(rope[:, 1::2])
    return np.ascontiguousarray(np.concatenate(cols, axis=1))


def _pack_wckv(Wckv):
    """[C, 576]: lat 512 | rope_even 32 | rope_odd 32."""
    lat = Wckv[:, :LORA]
    rope = Wckv[:, LORA:]
    return np.ascontiguousarray(
        np.concatenate([lat, rope[:, 0::2], rope[:, 1::2]], axis=1))


def _pack_wdkv(Wdkv, kv_norm_w, hg):
    """(wdn [LORA, 512], wdv [LORA, 512]) with kv_norm_w folded in."""
    Wd = Wdkv * kv_norm_w[:, None]
    heads = [4 * hg + i for i in range(HPC)]
    n_cols = [Wd[:, h * (NOPE + VD): h * (NOPE + VD) + NOPE] for h in heads]
    v_cols = [Wd[:, h * (NOPE + VD) + NOPE: (h + 1) * (NOPE + VD)]
              for h in heads]
    return (np.ascontiguousarray(np.concatenate(n_cols, axis=1)),
            np.ascontiguousarray(np.concatenate(v_cols, axis=1)))


# ---------------------------------------------------------------- program

def _act_tables_combined_only(arch):
    """Steer Bacc's ACT table chooser to the one set containing Exp+Ln+Copy
    so the kernel pays a single ~2.7us table load instead of thrashing
    between exp_and_others and natural_log on every softmax normalize."""
    from concourse.hw_specs import get_activation_tables
    mine = {"Exp", "Ln", "Copy", "Identity", "Square", "MemsetZero"}
    t = get_activation_tables(arch)
    out = {}
    for name, fns in t.items():
        if name == "natural_log_exp_and_others" or not any(
                f.name in mine for f in fns):
            out[name] = fns
        else:
            out[name] = set()
    return out


def _build(plan, n_generic):
    import concourse.mybir as mybir
    import concourse.tile as tile
    from concourse import bacc
    from concourse.masks import make_identity

    f32 = mybir.dt.float32
    f32r = mybir.dt.float32r
    mdt = getattr(mybir.dt, MM_DTYPE)
    AL = mybir.AluOpType
    AF = mybir.ActivationFunctionType

    def r(ap):
        return ap

    def zero_view(ap):
        return ap.bitcast(f32) if MM_DTYPE == "float32r" else ap

    nc = bacc.Bacc(None, target_bir_lowering=False)

    xT_d = nc.dram_tensor("xT", [C, T], mdt, kind="ExternalInput")
    wq_d = nc.dram_tensor("wq", [C, 768], mdt, kind="ExternalInput")
    wckv_d = nc.dram_tensor("wckv", [C, 576], mdt, kind="ExternalInput")
    wdn_d = nc.dram_tensor("wdn", [LORA, 512], mdt, kind="ExternalInput")
    wdv_d = nc.dram_tensor("wdv", [LORA, 512], mdt, kind="ExternalInput")
    wproj_d = nc.dram_tensor("wproj", [512, C], mdt, kind="ExternalInput")
    cos_d = nc.dram_tensor("cosT", [T, 32], f32, kind="ExternalInput")
    sin_d = nc.dram_tensor("sinT", [T, 32], f32, kind="ExternalInput")
    maskg_d = nc.dram_tensor("maskg", [max(1, n_generic), 128, 128], f32,
                             kind="ExternalInput")
    out_d = nc.dram_tensor("out", [T, C], f32, kind="ExternalOutput")

    with tile.TileContext(nc) as tc:
        with tc.tile_pool(name="const", bufs=1) as const, \
             tc.tile_pool(name="p1", bufs=1) as p1, \
             tc.tile_pool(name="pkn", bufs=1) as pkn, \
             tc.tile_pool(name="pqv", bufs=1) as pqv:
            ident_f = const.tile([P, P], f32, tag="ident_f", name="ident_f")
            make_identity(nc, ident_f)
            ident = const.tile([P, P], mdt, tag="ident", name="ident")
            nc.scalar.copy(ident, ident_f)
            ones_f = const.tile([P, P], f32, tag="ones_f", name="ones_f")
            nc.any.memset(ones_f, 1.0)
            ones_m = const.tile([P, P], mdt, tag="ones_m", name="ones_m")
            nc.scalar.copy(ones_m, ones_f)
            eps_t = const.tile([P, 1], f32, tag="eps", name="eps")
            nc.any.memset(eps_t, EPS)
            cos_all = const.tile([P, TB, 32], f32, tag="cos", name="cos")
            sin_all = const.tile([P, TB, 32], f32, tag="sin", name="sin")

            # zero-padded per-head q_rope^T: head h occupies rows
            # 64*(h%2) .. 64*(h%2)+64, the other 64 rows are zero so the
            # phase-D rope matmul can contract all 128 partitions against
            # the duplicated kropeT (K=64 matmuls stream ~2x slower).
            qropeTz = [p1.tile([P, T], mdt, tag=f"qrTz{h}", name=f"qrTz{h}")
                       for h in range(4)]
            kropeT = p1.tile([P, T], mdt, tag="krT", name="krT")
            knopeT = [pkn.tile([P, T], mdt, tag=f"knT{h}", name=f"knT{h}")
                      for h in range(4)]
            # SBUF-resident intermediates (no DRAM scratch round-trips)
            qnopeSB = pqv.tile([P, 4, T], mdt, tag="qnSB", name="qnSB")
            vSB = pqv.tile([P, TB, 512], mdt, tag="vSB", name="vSB")
            wproj_sb = [pqv.tile([P, C], mdt, tag=f"wp{h}", name=f"wp{h}")
                        for h in range(4)]
            xT_r = xT_d[:].rearrange("(kc p) t -> p kc t", p=P)

            # one PSUM pool for the whole kernel: per-bank tags make bank
            # reuse a per-slot WAR dep; no pool-boundary barriers between
            # phases
            psu_cm = tc.tile_pool(name="psu", bufs=1, space="PSUM")
            psu = psu_cm.__enter__()

            # pool for tensors that live through phases A+B only
            pab_cm = tc.tile_pool(name="pab", bufs=1)
            pab = pab_cm.__enter__()
            qrotSB = pab.tile([P, TB, 256], mdt, tag="qrotSB", name="qrotSB")
            wckv_sb = pab.tile([P, KC, 576], mdt, tag="wckv", name="wckv")
            wckv_r = wckv_d[:].rearrange("(kc p) m -> p kc m", p=P)

            # ---- phase A: q_nope^T and rotated q_rope -> SBUF ----
            with tc.tile_pool(name="phA", bufs=1) as pA, \
                 tc.tile_pool(name="xa", bufs=4) as xa_pool, \
                 tc.tile_pool(name="stA", bufs=3) as stA:
                wqn = pA.tile([P, KC, 512], mdt, tag="wqn", name="wqn")
                wqr = pA.tile([P, KC, 256], mdt, tag="wqr", name="wqr")
                wq_r = wq_d[:].rearrange("(kc p) m -> p kc m", p=P)
                nc.sync.dma_start(wqn[:, 0:4], wq_r[:, 0:4, 0:512])

                for n in range(4):
                    pn = [psu.tile([P, 512], f32, tag=f"bk{m}",
                                   name=f"qn{m}") for m in range(4)]
                    pr = [psu.tile([P, 256], f32, tag=f"bk{4 + s}",
                                   name=f"qr{s}") for s in range(4)]
                    # two bulk DMAs fetch all 16 xT chunks for this t-slice;
                    # each psum group then runs 16 back-to-back matmuls
                    xah = []
                    for hf in range(2):
                        xa = xa_pool.tile([P, 8, 512], mdt, tag="xa",
                                          name="xa")
                        nc.sync.dma_start(
                            xa, xT_r[:, 8 * hf:8 * hf + 8,
                                     n * 512:(n + 1) * 512])
                        xah.append(xa)

                    if n == 0:
                        # remaining weights queue behind the first-matmul
                        # critical loads, ordered by first compute use
                        for qk in range(1, 4):
                            nc.sync.dma_start(
                                wqn[:, 4 * qk:4 * qk + 4],
                                wq_r[:, 4 * qk:4 * qk + 4, 0:512])
                        nc.sync.dma_start(wqr, wq_r[:, :, 512:768])
                        nc.sync.dma_start(
                            cos_all,
                            cos_d[:].rearrange("(tb p) i -> p tb i", p=P))
                        nc.sync.dma_start(
                            sin_all,
                            sin_d[:].rearrange("(tb p) i -> p tb i", p=P))
                        # prefetch phase-B weights while phase A computes
                        for wk in range(4):
                            nc.sync.dma_start(
                                wckv_sb[:, 4 * wk:4 * wk + 4],
                                wckv_r[:, 4 * wk:4 * wk + 4])

                    def xat(k):
                        return xah[k // 8][:, k % 8]

                    for m in range(4):
                        for k in range(KC):
                            nc.tensor.matmul(
                                pn[m], r(wqn[:, k, m * 128:(m + 1) * 128]),
                                r(xat(k)), start=(k == 0), stop=(k == KC - 1))
                    for s in range(4):
                        for k in range(KC):
                            nc.tensor.matmul(
                                pr[s], r(xat(k)[:, s * 128:(s + 1) * 128]),
                                r(wqr[:, k, :]),
                                start=(k == 0), stop=(k == KC - 1))
                    for m in range(4):
                        nc.scalar.copy(
                            qnopeSB[:, m, n * 512:(n + 1) * 512], pn[m])
                    for s in range(4):
                        tb = n * 4 + s
                        cosv = cos_all[:, tb][:, None, :].to_broadcast(
                            (P, 4, 32))
                        sinv = sin_all[:, tb][:, None, :].to_broadcast(
                            (P, 4, 32))
                        prv = pr[s].rearrange("p (g i) -> p g i", i=64)
                        qe, qo = prv[:, :, 0:32], prv[:, :, 32:64]
                        ta = stA.tile([P, 128], f32, tag="ta",
                                      name="ta").rearrange(
                            "p (g i) -> p g i", i=32)
                        tb_ = stA.tile([P, 128], f32, tag="tb",
                                       name="tb").rearrange(
                            "p (g i) -> p g i", i=32)
                        tc2 = stA.tile([P, 128], f32, tag="tc",
                                       name="tc").rearrange(
                            "p (g i) -> p g i", i=32)
                        td = stA.tile([P, 128], f32, tag="td",
                                      name="td").rearrange(
                            "p (g i) -> p g i", i=32)
                        qvv = qrotSB[:, tb, :].rearrange(
                            "p (g i) -> p g i", i=64)
                        nc.vector.tensor_tensor(ta, qe, cosv, AL.mult)
                        nc.vector.tensor_tensor(tb_, qo, sinv, AL.mult)
                        nc.vector.tensor_tensor(qvv[:, :, 0:32], ta, tb_,
                                                AL.subtract)
                        nc.vector.tensor_tensor(tc2, qo, cosv, AL.mult)
                        nc.vector.tensor_tensor(td, qe, sinv, AL.mult)
                        nc.vector.tensor_tensor(qvv[:, :, 32:64], tc2, td,
                                                AL.add)

            # ---- phase B: ckv -> rms/rope -> transposed tensors ----------
            with tc.tile_pool(name="pckvT", bufs=1) as pckvT:
                ckvT = [pckvT.tile([P, T], mdt, tag=f"ckvT{d}",
                                   name=f"ckvT{d}") for d in range(4)]
                # phase-C weights: issue their loads before phase B compute
                wdn = pckvT.tile([P, 4, 512], mdt, tag="wdn", name="wdn")
                wdv = pckvT.tile([P, 4, 512], mdt, tag="wdv", name="wdv")
                nc.sync.dma_start(
                    wdn, wdn_d[:].rearrange("(kc p) m -> p kc m", p=P))
                nc.sync.dma_start(
                    wdv, wdv_d[:].rearrange("(kc p) m -> p kc m", p=P))

                # zero the dead half of each per-head q_rope^T tile once
                for h in range(4):
                    dead = slice(64, 128) if h % 2 == 0 else slice(0, 64)
                    nc.any.memset(zero_view(qropeTz[h][dead, :]), 0.0)

                with tc.tile_pool(name="xb", bufs=3) as xb_pool, \
                     tc.tile_pool(name="stB", bufs=3) as stB, \
                     tc.tile_pool(name="smB", bufs=4) as smB:
                    tr_idx = [0]
                    for n in range(8):  # 256-token slices
                        pcs = [[psu.tile([P, 288], f32,
                                         tag=f"bk{2 * s_ + u}",
                                         name=f"ckv{u}")
                                for u in range(2)] for s_ in range(2)]
                        xbh = []
                        for hf in range(2):
                            xb = xb_pool.tile([P, 8, 256], mdt, tag="xb",
                                              name="xb")
                            nc.sync.dma_start(
                                xb, xT_r[:, 8 * hf:8 * hf + 8,
                                         n * 256:(n + 1) * 256])
                            xbh.append(xb)

                        def xbt(k):
                            return xbh[k // 8][:, k % 8]

                        for s in range(2):
                            for u in range(2):
                                wsl = (slice(0, 288), slice(288, 576))[u]
                                for k in range(KC):
                                    nc.tensor.matmul(
                                        pcs[s][u],
                                        r(xbt(k)[:, s * 128:(s + 1) * 128]),
                                        r(wckv_sb[:, k, wsl]),
                                        start=(k == 0), stop=(k == KC - 1))
                        for s in range(2):
                            tb = n * 2 + s
                            p0, p1_ = pcs[s]
                            sq = stB.tile([P, 288], f32, tag="sq", name="sq")
                            sq2 = stB.tile([P, 224], f32, tag="sq2",
                                           name="sq2")
                            ss0 = smB.tile([P, 1], f32, tag="ss0", name="ss0")
                            ss1 = smB.tile([P, 1], f32, tag="ss1", name="ss1")
                            nc.scalar.activation(sq, p0, AF.Square,
                                                 accum_out=ss0)
                            nc.scalar.activation(sq2, p1_[:, 0:224],
                                                 AF.Square, accum_out=ss1)
                            ssum = smB.tile([P, 1], f32, tag="ss", name="ss")
                            nc.vector.tensor_add(ssum, ss0, ss1)
                            lnv = smB.tile([P, 1], f32, tag="lnv", name="lnv")
                            nc.scalar.activation(lnv, ssum, AF.Ln,
                                                 bias=eps_t,
                                                 scale=1.0 / LORA)
                            rfac = smB.tile([P, 1], f32, tag="rfac",
                                            name="rfac")
                            nc.scalar.activation(rfac, lnv, AF.Exp,
                                                 scale=-0.5)
                            ckvn = stB.tile([P, 512], mdt, tag="ckvn",
                                            name="ckvn")
                            nc.scalar.mul(ckvn[:, 0:288], p0, rfac)
                            nc.scalar.mul(ckvn[:, 288:512], p1_[:, 0:224],
                                          rfac)
                            # k_rope rotation (raw latent, un-normalized)
                            ke, ko = p1_[:, 224:256], p1_[:, 256:288]
                            cosv, sinv = cos_all[:, tb], sin_all[:, tb]
                            ra = stB.tile([P, 32], f32, tag="ra", name="ra")
                            rb = stB.tile([P, 32], f32, tag="rb", name="rb")
                            rc = stB.tile([P, 32], f32, tag="rc", name="rc")
                            rd = stB.tile([P, 32], f32, tag="rd", name="rd")
                            krt = stB.tile([P, 64], mdt, tag="krt",
                                           name="krt")
                            nc.vector.tensor_tensor(ra, ke, cosv, AL.mult)
                            nc.vector.tensor_tensor(rb, ko, sinv, AL.mult)
                            nc.vector.tensor_tensor(krt[:, 0:32], ra, rb,
                                                    AL.subtract)
                            nc.vector.tensor_tensor(rc, ko, cosv, AL.mult)
                            nc.vector.tensor_tensor(rd, ke, sinv, AL.mult)
                            nc.vector.tensor_tensor(krt[:, 32:64], rc, rd,
                                                    AL.add)
                            # transposes -> persistent ^T tensors
                            tcol = slice(tb * 128, (tb + 1) * 128)
                            for dc in range(4):
                                pt = psu.tile([P, P], mdt,
                                              tag=f"bk{4 + tr_idx[0] % 4}",
                                              name="tr")
                                tr_idx[0] += 1
                                nc.tensor.transpose(
                                    pt, ckvn[:, dc * 128:(dc + 1) * 128],
                                    ident)
                                nc.vector.tensor_copy(ckvT[dc][:, tcol], pt)
                            pt = psu.tile([P, P], mdt,
                                          tag=f"bk{4 + tr_idx[0] % 4}",
                                          name="tr")
                            tr_idx[0] += 1
                            nc.tensor.transpose(pt[0:64, :], krt, ident)
                            nc.vector.tensor_copy(kropeT[0:64, tcol],
                                                  pt[0:64, :])
                            for pc in range(2):
                                pt = psu.tile([P, P], mdt,
                                              tag=f"bk{4 + tr_idx[0] % 4}",
                                              name="tr")
                                tr_idx[0] += 1
                                nc.tensor.transpose(
                                    pt,
                                    qrotSB[:, tb, pc * 128:(pc + 1) * 128],
                                    ident)
                                # split the head pair into zero-padded
                                # per-head tiles
                                he, ho = 2 * pc, 2 * pc + 1
                                nc.vector.tensor_copy(
                                    qropeTz[he][0:64, tcol], pt[0:64, :])
                                nc.vector.tensor_copy(
                                    qropeTz[ho][64:128, tcol], pt[64:128, :])

                # duplicate k_rope^T rows for the padded rope matmuls
                nc.sync.dma_start(kropeT[64:128, :], kropeT[0:64, :])

                # ---- phase C: k_nope^T per head, v -> SBUF ------
                # prefetch phase-D proj weights during phase C compute
                for h in range(4):
                    nc.sync.dma_start(wproj_sb[h],
                                      wproj_d[h * P:(h + 1) * P, :])
                for h in range(4):
                    for n4 in range(4):
                        pk = psu.tile([P, 512], f32,
                                      tag=f"bk{(h * 4 + n4) % 2}",
                                      name="kn")
                        for kc in range(4):
                            nc.tensor.matmul(
                                pk, r(wdn[:, kc, h * 128:(h + 1) * 128]),
                                r(ckvT[kc][:, n4 * 512:(n4 + 1) * 512]),
                                start=(kc == 0), stop=(kc == 3))
                        nc.vector.tensor_copy(
                            knopeT[h][:, n4 * 512:(n4 + 1) * 512], pk)
                for tb in range(TB):
                    pv = psu.tile([P, 512], f32,
                                  tag=f"bk{2 + tb % 2}", name="v")
                    for kc in range(4):
                        nc.tensor.matmul(
                            pv, r(ckvT[kc][:, tb * P:(tb + 1) * P]),
                            r(wdv[:, kc, :]),
                            start=(kc == 0), stop=(kc == 3))
                    nc.vector.tensor_copy(vSB[:, tb, :], pv)

            pab_cm.__exit__(None, None, None)

            # ---- phase D: attention + proj -------------------------------
            with tc.tile_pool(name="mgp", bufs=8) as mgp, \
                 tc.tile_pool(name="sp", bufs=8) as sp, \
                 tc.tile_pool(name="spa", bufs=2) as spa, \
                 tc.tile_pool(name="stD", bufs=3) as stD, \
                 tc.tile_pool(name="attp", bufs=2) as attp:
                for j in range(NQ):
                    chunks = plan[j]
                    nchunks = len(chunks)
                    mg_tiles = {}
                    for (c, col0, sub) in chunks:
                        for s in sub:
                            if not isinstance(s, str):
                                mt = mgp.tile([P, P], f32, tag="mg",
                                              name="mg")
                                nc.sync.dma_start(mt, maskg_d[s[1]])
                                mg_tiles[s[1]] = mt
                    attT = {}
                    for g in range(4):  # heads, pipelined sequentially
                        h = g
                        qn = qnopeSB[:, h, j * 512:(j + 1) * 512]
                        ps_att = psu.tile([P, 512], f32,
                                          tag=f"bk{4 + g % 2}", name="att")
                        spacc = spa.tile([P, 512], f32, tag="spacc",
                                         name="spacc")

                        def scores_mm(ci):
                            c, col0, sub = chunks[ci]
                            qsl = slice(512 * j + col0, 512 * (j + 1))
                            kcl = slice(128 * c, 128 * (c + 1))
                            ps_s = psu.tile([P, 512], f32,
                                            tag=f"bk{ci % 3}", name="s")
                            nc.tensor.matmul(
                                ps_s[:, col0:], r(knopeT[h][:, kcl]),
                                r(qn[:, col0:]),
                                start=True, stop=False)
                            nc.tensor.matmul(
                                ps_s[:, col0:], r(kropeT[:, kcl]),
                                r(qropeTz[h][:, qsl]),
                                start=False, stop=True)
                            return ps_s

                        def exp_mask(ci, ps_s):
                            c, col0, sub = chunks[ci]
                            sprime = sp.tile([P, 512], mdt, tag="sp",
                                             name="sp")
                            nc.scalar.activation(
                                sprime[:, col0:], ps_s[:, col0:],
                                AF.Exp, scale=SCALE)
                            for qs, s in enumerate(sub):
                                colA, colB = 128 * qs, 128 * (qs + 1)
                                if colA < col0 or s == "zero":
                                    continue
                                if s == "skip":
                                    nc.any.memset(
                                        zero_view(sprime[:, colA:colB]), 0.0)
                                else:
                                    mt = mg_tiles[s[1]]
                                    stt = stD.tile([P, P], f32, tag="stt",
                                                   name="stt")
                                    nc.vector.scalar_tensor_tensor(
                                        stt, ps_s[:, colA:colB], SCALE, mt,
                                        AL.mult, AL.add)
                                    nc.scalar.activation(
                                        sprime[:, colA:colB], stt, AF.Exp,
                                        scale=1.0)
                            return sprime

                        def acc_mm(ci, sprime):
                            # on GpSimd: keeps the DVE queue free for the
                            # STT/normalize/psum-drain ops phase D needs
                            c, col0, sub = chunks[ci]
                            if ci == 0:
                                nc.gpsimd.tensor_copy(spacc, sprime)
                            else:
                                nc.gpsimd.tensor_tensor(
                                    spacc[:, col0:], spacc[:, col0:],
                                    sprime[:, col0:], AL.add)

                        def att_mm(ci, sprime):
                            c, col0, sub = chunks[ci]
                            nc.tensor.matmul(
                                ps_att[:, col0:],
                                r(vSB[:, c, h * 128:(h + 1) * 128]),
                                r(sprime[:, col0:]),
                                start=(ci == 0), stop=(ci == nchunks - 1))

                        pend = [scores_mm(0)]
                        for pi in range(1, min(3, nchunks)):
                            pend.append(scores_mm(pi))
                        for ci in range(nchunks):
                            sprime = exp_mask(ci, pend[ci])
                            if ci + 3 < nchunks:
                                pend.append(scores_mm(ci + 3))
                            acc_mm(ci, sprime)
                            att_mm(ci, sprime)
                        # row-sum l: round the DVE-accumulated fp32 sprime
                        # sum to the matmul dtype, then one ones-matmul
                        spacc_m = spa.tile([P, 512], mdt, tag="spacc_m",
                                           name="spacc_m")
                        if MM_DTYPE == "float32r":
                            nc.scalar.copy(spacc_m, spacc)
                        else:
                            nc.gpsimd.tensor_copy(spacc_m, spacc)
                        ps_l = psu.tile([P, 512], f32, tag="bk3", name="l")
                        nc.tensor.matmul(ps_l, r(ones_m), r(spacc_m),
                                         start=True, stop=True)
                        lnl = stD.tile([P, 512], f32, tag="lr", name="lr")
                        nc.scalar.activation(lnl, ps_l, AF.Ln)
                        rec = stD.tile([P, 512], f32, tag="lr", name="lr")
                        nc.scalar.activation(rec, lnl, AF.Exp, scale=-1.0)
                        at = attp.tile([P, 512], mdt, tag=f"at{h}",
                                       name=f"at{h}")
                        nc.vector.tensor_tensor(at, ps_att, rec, AL.mult)
                        attT[h] = at
                    for qs in range(4):
                        for ct in range(4):
                            pso = psu.tile([P, 512], f32,
                                           tag=f"bk{6 + ct % 2}", name="o")
                            for h in range(4):
                                nc.tensor.matmul(
                                    pso,
                                    r(attT[h][:, qs * 128:(qs + 1) * 128]),
                                    r(wproj_sb[h][:,
                                                  ct * 512:(ct + 1) * 512]),
                                    start=(h == 0), stop=(h == 3))
                            ost = sp.tile([P, 512], f32, tag="ost",
                                          name="ost", bufs=3)
                            nc.vector.tensor_copy(ost, pso)
                            nc.sync.dma_start(
                                out_d[512 * j + 128 * qs:
                                      512 * j + 128 * (qs + 1),
                                      ct * 512:(ct + 1) * 512], ost)

            psu_cm.__exit__(None, None, None)

    orig_tables = bacc.get_activation_tables
    bacc.get_activation_tables = _act_tables_combined_only
    try:
        nc.compile()
    finally:
        bacc.get_activation_tables = orig_tables
    return nc


# ---------------------------------------------------------------- entry

def _ensure_axon_hook_shim():
    # bass_utils imports antenv.axon_hooks when tracing is requested via
    # env; provide a null hook module if the image lacks it so kernel()
    # never crashes on that path.
    try:
        import antenv.axon_hooks  # noqa: F401
    except Exception:
        import sys
        import types
        m = types.ModuleType("antenv.axon_hooks")
        _h = [None]
        m.set_axon_ntff_profile_hook = lambda h: _h.__setitem__(0, h)
        m.get_axon_ntff_profile_hook = lambda: _h[0]
        sys.modules["antenv.axon_hooks"] = m
        try:
            import antenv
            antenv.axon_hooks = m
        except Exception:
            pass


def kernel(x, freq_cis, mask, window, Wq, Wckv, kv_norm_w, Wdkv, Wproj,
           start_pos):
    global LAST_RESULTS
    _ensure_axon_hook_shim()
    from concourse.bass_utils import run_bass_kernel_spmd

    x = np.asarray(x, np.float32)
    freq_cis = np.asarray(freq_cis, np.float32)
    mask = np.asarray(mask, np.float32)
    Wq = np.asarray(Wq, np.float32)
    Wckv = np.asarray(Wckv, np.float32)
    kv_norm_w = np.asarray(kv_norm_w, np.float32)
    Wdkv = np.asarray(Wdkv, np.float32)
    Wproj = np.asarray(Wproj, np.float32)

    plan, maskg = _mask_plan(mask)
    key = (MM_DTYPE, _plan_key(plan))
    if key not in _prog_cache:
        _prog_cache[key] = _build(plan, maskg.shape[0])
    nc = _prog_cache[key]

    cosT = np.ascontiguousarray(freq_cis[:, :, 0])
    sinT = np.ascontiguousarray(freq_cis[:, :, 1])
    wckv_p = _pack_wckv(Wckv)

    in_maps = []
    for core in range(N_CORES):
        b, hg = core // 4, core % 4
        wdn, wdv = _pack_wdkv(Wdkv, kv_norm_w, hg)
        in_maps.append({
            "xT": np.ascontiguousarray(x[b].T),
            "wq": _pack_wq(Wq, hg),
            "wckv": wckv_p,
            "wdn": wdn,
            "wdv": wdv,
            "wproj": np.ascontiguousarray(Wproj[hg * 512:(hg + 1) * 512, :]),
            "cosT": cosT,
            "sinT": sinT,
            "maskg": maskg,
        })

    if MM_DTYPE == "bfloat16":
        import ml_dtypes
        mmdt = ml_dtypes.bfloat16
        for m in in_maps:
            for k in ("xT", "wq", "wckv", "wdn", "wdv", "wproj"):
                m[k] = m[k].astype(mmdt)

    res = run_bass_kernel_spmd(nc, in_maps, list(range(N_CORES)))
    LAST_RESULTS = res
    outs = [res.results[c]["out"] for c in range(N_CORES)]
    full = np.empty((B, T, C), np.float32)
    for b in range(B):
        full[b] = outs[4 * b] + outs[4 * b + 1] + outs[4 * b + 2] \
            + outs[4 * b + 3]
    return full


# revision 12
# speedup vs baseline: 1.1506x; 1.1506x over previous
"""MLA prefill kernel for 8 trn2 NeuronCores.

Sharding: core c handles batch b = c//4, head group hg = c%4 (4 of 16 heads).
Each core computes its 4 heads' attention + its partial proj output
[T, C]; the host sums the 4 partials per batch (unshard of the
head-contracted proj output) and stacks batches.

Device dataflow per core (matmuls in bf16, psum fp32):
  A: q_nope^T [4x128, T] -> SBUF-resident qnopeSB; q_rope [t,d] rotated
     -> SBUF-resident qrotSB
  B: ckv [t, 576] -> rmsnorm(lat), rotate k_rope, PE-transpose to
     ckv'^T [4x128, T], kropeT(dup) [128, T], qropeTz [4x(128, T)]
     zero-padded per head so phase D rope matmuls contract K=128
  C: k_nope^T per head [128, T], v [t, 4*128] -> SBUF-resident vSB
  D: per (head, q-block 512): S^T tiles [128k, 512q] (nope+rope matmuls),
     exp via ACT from PSUM (masked subblocks via DVE STT), row-sum l via
     DVE accumulation of sprime chunks + one fp32r ones-matmul,
     att^T accum via V-matmul; normalize by exp(-ln l); proj accumulates
     4 head-chunks -> direct PSUM->HBM DMA.
  One PSUM pool with per-bank tags spans all phases (no pool barriers).
"""

import numpy as np

B, T, C, H = 2, 2048, 2048, 16
NOPE, ROPE, VD, LORA = 128, 64, 128, 512
QK = NOPE + ROPE
EPS = 1e-6
SCALE = 1.0 / float(np.sqrt(QK))
P = 128
KC = C // P    # 16 contraction chunks over C
TB = T // P    # 16 token sub-blocks
NQ = T // 512  # 4 q-blocks
HPC = 4        # heads per core
N_CORES = 8

_prog_cache = {}
LAST_RESULTS = None  # BassKernelResults of the most recent run (for test.py)
MM_DTYPE = "bfloat16"  # "bfloat16" or "float32r" for all matmul operands


# ---------------------------------------------------------------- host prep

def _mask_plan(mask):
    """Classify mask into per-(q-block, k-chunk) plans.

    plan[j] = list of (c, col0, subops); subops[qs] in
    {"skip", "zero", ("g", gidx)} for columns [128*qs, 128*qs+128) of the
    S^T tile. col0 = 128 * (# leading skip sub-blocks), forced to 0 for
    the first chunk of each j. Chunks with all sub-blocks skip are
    omitted (their softmax contribution is exactly 0 in fp32).
    """
    plan = []
    gblocks = []
    for j in range(NQ):
        chunks = []
        for c in range(TB):
            sub = []
            nskip_lead = 0
            leading = True
            any_alive = False
            for qs in range(4):
                blk = mask[512 * j + 128 * qs: 512 * j + 128 * qs + 128,
                           128 * c: 128 * c + 128]
                if np.all(blk <= -88.0):
                    sub.append("skip")
                    if leading:
                        nskip_lead += 1
                elif np.all(blk == 0.0):
                    sub.append("zero")
                    leading = False
                    any_alive = True
                else:
                    gidx = len(gblocks)
                    gblocks.append(np.ascontiguousarray(blk.T))
                    sub.append(("g", gidx))
                    leading = False
                    any_alive = True
            if not any_alive:
                continue
            col0 = 128 * nskip_lead
            if not chunks:
                col0 = 0  # first chunk must initialize full psum width
            chunks.append((c, col0, sub))
        assert chunks, f"q-block {j}: all keys masked (unsupported)"
        plan.append(chunks)
    if gblocks:
        garr = np.stack(gblocks).astype(np.float32)
    else:
        garr = np.zeros((1, 128, 128), np.float32)
    return plan, garr


def _plan_key(plan):
    return tuple(
        tuple((c, col0, tuple(s if isinstance(s, str) else ("g",) for s in sub))
              for (c, col0, sub) in chunks)
        for chunks in plan
    )


def _pack_wq(Wq, hg):
    """[C, 768]: 4 heads' nope cols, then 2 rope 'pair' chunks laid out
    [h_even(32) h_odd(32) h'_even(32) h'_odd(32)]."""
    heads = [4 * hg + i for i in range(HPC)]
    cols = [Wq[:, h * QK: h * QK + NOPE] for h in heads]
    for h in heads:
        rope = Wq[:, h * QK + NOPE: h * QK + QK]
        cols.append(rope[:, 0::2])
        cols.append(rope[:, 1::2])
    return np.ascontiguousarray(np.concatenate(cols, axis=1))


def _pack_wckv(Wckv):
    """[C, 576]: lat 512 | rope_even 32 | rope_odd 32."""
    lat = Wckv[:, :LORA]
    rope = Wckv[:, LORA:]
    return np.ascontiguousarray(
        np.concatenate([lat, rope[:, 0::2], rope[:, 1::2]], axis=1))


def _pack_wdkv(Wdkv, kv_norm_w, hg):
    """(wdn [LORA, 512], wdv [LORA, 512]) with kv_norm_w folded in."""
    Wd = Wdkv * kv_norm_w[:, None]
    heads = [4 * hg + i for i in range(HPC)]
    n_cols = [Wd[:, h * (NOPE + VD): h * (NOPE + VD) + NOPE] for h in heads]
    v_cols = [Wd[:, h * (NOPE + VD) + NOPE: (h + 1) * (NOPE + VD)]
              for h in heads]
    return (np.ascontiguousarray(np.concatenate(n_cols, axis=1)),
            np.ascontiguousarray(np.concatenate(v_cols, axis=1)))


# ---------------------------------------------------------------- program

def _act_tables_combined_only(arch):
    """Steer Bacc's ACT table chooser to the one set containing Exp+Ln+Copy
    so the kernel pays a single ~2.7us table load instead of thrashing
    between exp_and_others and natural_log on every softmax normalize."""
    from concourse.hw_specs import get_activation_tables
    mine = {"Exp", "Ln", "Copy", "Identity", "Square", "MemsetZero"}
    t = get_activation_tables(arch)
    out = {}
    for name, fns in t.items():
        if name == "natural_log_exp_and_others" or not any(
                f.name in mine for f in fns):
            out[name] = fns
        else:
            out[name] = set()
    return out


def _build(plan, n_generic):
    import concourse.mybir as mybir
    import concourse.tile as tile
    from concourse import bacc
    from concourse.masks import make_identity

    f32 = mybir.dt.float32
    f32r = mybir.dt.float32r
    mdt = getattr(mybir.dt, MM_DTYPE)
    AL = mybir.AluOpType
    AF = mybir.ActivationFunctionType

    def r(ap):
        return ap

    def zero_view(ap):
        return ap.bitcast(f32) if MM_DTYPE == "float32r" else ap

    nc = bacc.Bacc(None, target_bir_lowering=False)

    xT_d = nc.dram_tensor("xT", [C, T], mdt, kind="ExternalInput")
    wq_d = nc.dram_tensor("wq", [C, 768], mdt, kind="ExternalInput")
    wckv_d = nc.dram_tensor("wckv", [C, 576], mdt, kind="ExternalInput")
    wdn_d = nc.dram_tensor("wdn", [LORA, 512], mdt, kind="ExternalInput")
    wdv_d = nc.dram_tensor("wdv", [LORA, 512], mdt, kind="ExternalInput")
    wproj_d = nc.dram_tensor("wproj", [512, C], mdt, kind="ExternalInput")
    cos_d = nc.dram_tensor("cosT", [T, 32], f32, kind="ExternalInput")
    sin_d = nc.dram_tensor("sinT", [T, 32], f32, kind="ExternalInput")
    maskg_d = nc.dram_tensor("maskg", [max(1, n_generic), 128, 128], f32,
                             kind="ExternalInput")
    out_d = nc.dram_tensor("out", [T, C], f32, kind="ExternalOutput")

    with tile.TileContext(nc) as tc:
        with tc.tile_pool(name="const", bufs=1) as const, \
             tc.tile_pool(name="p1", bufs=1) as p1, \
             tc.tile_pool(name="pkn", bufs=1) as pkn, \
             tc.tile_pool(name="pqv", bufs=1) as pqv:
            ident_f = const.tile([P, P], f32, tag="ident_f", name="ident_f")
            make_identity(nc, ident_f)
            ident = const.tile([P, P], mdt, tag="ident", name="ident")
            nc.scalar.copy(ident, ident_f)
            ones_f = const.tile([P, P], f32, tag="ones_f", name="ones_f")
            nc.any.memset(ones_f, 1.0)
            ones_m = const.tile([P, P], mdt, tag="ones_m", name="ones_m")
            nc.scalar.copy(ones_m, ones_f)
            eps_t = const.tile([P, 1], f32, tag="eps", name="eps")
            nc.any.memset(eps_t, EPS)
            cos_all = const.tile([P, TB, 32], f32, tag="cos", name="cos")
            sin_all = const.tile([P, TB, 32], f32, tag="sin", name="sin")

            # zero-padded per-head q_rope^T: head h occupies rows
            # 64*(h%2) .. 64*(h%2)+64, the other 64 rows are zero so the
            # phase-D rope matmul can contract all 128 partitions against
            # the duplicated kropeT (K=64 matmuls stream ~2x slower).
            qropeTz = [p1.tile([P, T], mdt, tag=f"qrTz{h}", name=f"qrTz{h}")
                       for h in range(4)]
            kropeT = p1.tile([P, T], mdt, tag="krT", name="krT")
            knopeT = [pkn.tile([P, T], mdt, tag=f"knT{h}", name=f"knT{h}")
                      for h in range(4)]
            # SBUF-resident intermediates (no DRAM scratch round-trips)
            qnopeSB = pqv.tile([P, 4, T], mdt, tag="qnSB", name="qnSB")
            vSB = pqv.tile([P, TB, 512], mdt, tag="vSB", name="vSB")
            wproj_sb = [pqv.tile([P, C], mdt, tag=f"wp{h}", name=f"wp{h}")
                        for h in range(4)]
            xT_r = xT_d[:].rearrange("(kc p) t -> p kc t", p=P)

            # one PSUM pool for the whole kernel: per-bank tags make bank
            # reuse a per-slot WAR dep; no pool-boundary barriers between
            # phases
            psu_cm = tc.tile_pool(name="psu", bufs=1, space="PSUM")
            psu = psu_cm.__enter__()

            # pool for tensors that live through phases A+B only
            pab_cm = tc.tile_pool(name="pab", bufs=1)
            pab = pab_cm.__enter__()
            qrotSB = pab.tile([P, TB, 256], mdt, tag="qrotSB", name="qrotSB")
            wckv_sb = pab.tile([P, KC, 576], mdt, tag="wckv", name="wckv")
            wckv_r = wckv_d[:].rearrange("(kc p) m -> p kc m", p=P)

            # ---- phase A: q_nope^T and rotated q_rope -> SBUF ----
            with tc.tile_pool(name="phA", bufs=1) as pA, \
                 tc.tile_pool(name="xa", bufs=4) as xa_pool, \
                 tc.tile_pool(name="stA", bufs=3) as stA:
                wqn = pA.tile([P, KC, 512], mdt, tag="wqn", name="wqn")
                wqr = pA.tile([P, KC, 256], mdt, tag="wqr", name="wqr")
                wq_r = wq_d[:].rearrange("(kc p) m -> p kc m", p=P)
                nc.sync.dma_start(wqn[:, 0:4], wq_r[:, 0:4, 0:512])

                for n in range(4):
                    pn = [psu.tile([P, 512], f32, tag=f"bk{m}",
                                   name=f"qn{m}") for m in range(4)]
                    pr = [psu.tile([P, 256], f32, tag=f"bk{4 + s}",
                                   name=f"qr{s}") for s in range(4)]
                    # two bulk DMAs fetch all 16 xT chunks for this t-slice;
                    # each psum group then runs 16 back-to-back matmuls
                    xah = []
                    for hf in range(2):
                        xa = xa_pool.tile([P, 8, 512], mdt, tag="xa",
                                          name="xa")
                        nc.sync.dma_start(
                            xa, xT_r[:, 8 * hf:8 * hf + 8,
                                     n * 512:(n + 1) * 512])
                        xah.append(xa)

                    if n == 0:
                        # remaining weights queue behind the first-matmul
                        # critical loads, ordered by first compute use
                        for qk in range(1, 4):
                            nc.sync.dma_start(
                                wqn[:, 4 * qk:4 * qk + 4],
                                wq_r[:, 4 * qk:4 * qk + 4, 0:512])
                        nc.sync.dma_start(wqr, wq_r[:, :, 512:768])
                        nc.sync.dma_start(
                            cos_all,
                            cos_d[:].rearrange("(tb p) i -> p tb i", p=P))
                        nc.sync.dma_start(
                            sin_all,
                            sin_d[:].rearrange("(tb p) i -> p tb i", p=P))
                        # prefetch phase-B weights while phase A computes
                        for wk in range(4):
                            nc.sync.dma_start(
                                wckv_sb[:, 4 * wk:4 * wk + 4],
                                wckv_r[:, 4 * wk:4 * wk + 4])

                    def xat(k):
                        return xah[k // 8][:, k % 8]

                    for m in range(4):
                        for k in range(KC):
                            nc.tensor.matmul(
                                pn[m], r(wqn[:, k, m * 128:(m + 1) * 128]),
                                r(xat(k)), start=(k == 0), stop=(k == KC - 1))
                    for s in range(4):
                        for k in range(KC):
                            nc.tensor.matmul(
                                pr[s], r(xat(k)[:, s * 128:(s + 1) * 128]),
                                r(wqr[:, k, :]),
                                start=(k == 0), stop=(k == KC - 1))
                    for m in range(4):
                        nc.scalar.copy(
                            qnopeSB[:, m, n * 512:(n + 1) * 512], pn[m])
                    for s in range(4):
                        tb = n * 4 + s
                        cosv = cos_all[:, tb][:, None, :].to_broadcast(
                            (P, 4, 32))
                        sinv = sin_all[:, tb][:, None, :].to_broadcast(
                            (P, 4, 32))
                        prv = pr[s].rearrange("p (g i) -> p g i", i=64)
                        qe, qo = prv[:, :, 0:32], prv[:, :, 32:64]
                        ta = stA.tile([P, 128], f32, tag="ta",
                                      name="ta").rearrange(
                            "p (g i) -> p g i", i=32)
                        tb_ = stA.tile([P, 128], f32, tag="tb",
                                       name="tb").rearrange(
                            "p (g i) -> p g i", i=32)
                        tc2 = stA.tile([P, 128], f32, tag="tc",
                                       name="tc").rearrange(
                            "p (g i) -> p g i", i=32)
                        td = stA.tile([P, 128], f32, tag="td",
                                      name="td").rearrange(
                            "p (g i) -> p g i", i=32)
                        qvv = qrotSB[:, tb, :].rearrange(
                            "p (g i) -> p g i", i=64)
                        nc.vector.tensor_tensor(ta, qe, cosv, AL.mult)
                        nc.vector.tensor_tensor(tb_, qo, sinv, AL.mult)
                        nc.vector.tensor_tensor(qvv[:, :, 0:32], ta, tb_,
                                                AL.subtract)
                        nc.vector.tensor_tensor(tc2, qo, cosv, AL.mult)
                        nc.vector.tensor_tensor(td, qe, sinv, AL.mult)
                        nc.vector.tensor_tensor(qvv[:, :, 32:64], tc2, td,
                                                AL.add)

            # ---- phase B: ckv -> rms/rope -> transposed tensors ----------
            with tc.tile_pool(name="pckvT", bufs=1) as pckvT:
                ckvT = [pckvT.tile([P, T], mdt, tag=f"ckvT{d}",
                                   name=f"ckvT{d}") for d in range(4)]
                # phase-C weights: issue their loads before phase B compute
                wdn = pckvT.tile([P, 4, 512], mdt, tag="wdn", name="wdn")
                wdv = pckvT.tile([P, 4, 512], mdt, tag="wdv", name="wdv")
                nc.sync.dma_start(
                    wdn, wdn_d[:].rearrange("(kc p) m -> p kc m", p=P))
                nc.sync.dma_start(
                    wdv, wdv_d[:].rearrange("(kc p) m -> p kc m", p=P))

                # zero the dead half of each per-head q_rope^T tile once
                for h in range(4):
                    dead = slice(64, 128) if h % 2 == 0 else slice(0, 64)
                    nc.any.memset(zero_view(qropeTz[h][dead, :]), 0.0)

                with tc.tile_pool(name="xb", bufs=3) as xb_pool, \
                     tc.tile_pool(name="stB", bufs=3) as stB, \
                     tc.tile_pool(name="smB", bufs=4) as smB:
                    tr_idx = [0]
                    for n in range(8):  # 256-token slices
                        pcs = [[psu.tile([P, 288], f32,
                                         tag=f"bk{2 * s_ + u}",
                                         name=f"ckv{u}")
                                for u in range(2)] for s_ in range(2)]
                        xbh = []
                        for hf in range(2):
                            xb = xb_pool.tile([P, 8, 256], mdt, tag="xb",
                                              name="xb")
                            nc.sync.dma_start(
                                xb, xT_r[:, 8 * hf:8 * hf + 8,
                                         n * 256:(n + 1) * 256])
                            xbh.append(xb)

                        def xbt(k):
                            return xbh[k // 8][:, k % 8]

                        for s in range(2):
                            for u in range(2):
                                wsl = (slice(0, 288), slice(288, 576))[u]
                                for k in range(KC):
                                    nc.tensor.matmul(
                                        pcs[s][u],
                                        r(xbt(k)[:, s * 128:(s + 1) * 128]),
                                        r(wckv_sb[:, k, wsl]),
                                        start=(k == 0), stop=(k == KC - 1))
                        for s in range(2):
                            tb = n * 2 + s
                            p0, p1_ = pcs[s]
                            sq = stB.tile([P, 288], f32, tag="sq", name="sq")
                            sq2 = stB.tile([P, 224], f32, tag="sq2",
                                           name="sq2")
                            ss0 = smB.tile([P, 1], f32, tag="ss0", name="ss0")
                            ss1 = smB.tile([P, 1], f32, tag="ss1", name="ss1")
                            nc.scalar.activation(sq, p0, AF.Square,
                                                 accum_out=ss0)
                            nc.scalar.activation(sq2, p1_[:, 0:224],
                                                 AF.Square, accum_out=ss1)
                            ssum = smB.tile([P, 1], f32, tag="ss", name="ss")
                            nc.vector.tensor_add(ssum, ss0, ss1)
                            lnv = smB.tile([P, 1], f32, tag="lnv", name="lnv")
                            nc.scalar.activation(lnv, ssum, AF.Ln,
                                                 bias=eps_t,
                                                 scale=1.0 / LORA)
                            rfac = smB.tile([P, 1], f32, tag="rfac",
                                            name="rfac")
                            nc.scalar.activation(rfac, lnv, AF.Exp,
                                                 scale=-0.5)
                            ckvn = stB.tile([P, 512], mdt, tag="ckvn",
                                            name="ckvn")
                            nc.scalar.mul(ckvn[:, 0:288], p0, rfac)
                            nc.scalar.mul(ckvn[:, 288:512], p1_[:, 0:224],
                                          rfac)
                            # k_rope rotation (raw latent, un-normalized)
                            ke, ko = p1_[:, 224:256], p1_[:, 256:288]
                            cosv, sinv = cos_all[:, tb], sin_all[:, tb]
                            ra = stB.tile([P, 32], f32, tag="ra", name="ra")
                            rb = stB.tile([P, 32], f32, tag="rb", name="rb")
                            rc = stB.tile([P, 32], f32, tag="rc", name="rc")
                            rd = stB.tile([P, 32], f32, tag="rd", name="rd")
                            krt = stB.tile([P, 64], mdt, tag="krt",
                                           name="krt")
                            nc.vector.tensor_tensor(ra, ke, cosv, AL.mult)
                            nc.vector.tensor_tensor(rb, ko, sinv, AL.mult)
                            nc.vector.tensor_tensor(krt[:, 0:32], ra, rb,
                                                    AL.subtract)
                            nc.vector.tensor_tensor(rc, ko, cosv, AL.mult)
                            nc.vector.tensor_tensor(rd, ke, sinv, AL.mult)
                            nc.vector.tensor_tensor(krt[:, 32:64], rc, rd,
                                                    AL.add)
                            # transposes -> persistent ^T tensors
                            tcol = slice(tb * 128, (tb + 1) * 128)
                            for dc in range(4):
                                pt = psu.tile([P, P], mdt,
                                              tag=f"bk{4 + tr_idx[0] % 4}",
                                              name="tr")
                                tr_idx[0] += 1
                                nc.tensor.transpose(
                                    pt, ckvn[:, dc * 128:(dc + 1) * 128],
                                    ident)
                                nc.vector.tensor_copy(ckvT[dc][:, tcol], pt)
                            pt = psu.tile([P, P], mdt,
                                          tag=f"bk{4 + tr_idx[0] % 4}",
                                          name="tr")
                            tr_idx[0] += 1
                            nc.tensor.transpose(pt[0:64, :], krt, ident)
                            nc.vector.tensor_copy(kropeT[0:64, tcol],
                                                  pt[0:64, :])
                            for pc in range(2):
                                pt = psu.tile([P, P], mdt,
                                              tag=f"bk{4 + tr_idx[0] % 4}",
                                              name="tr")
                                tr_idx[0] += 1
                                nc.tensor.transpose(
                                    pt,
                                    qrotSB[:, tb, pc * 128:(pc + 1) * 128],
                                    ident)
                                # split the head pair into zero-padded
                                # per-head tiles
                                he, ho = 2 * pc, 2 * pc + 1
                                nc.vector.tensor_copy(
                                    qropeTz[he][0:64, tcol], pt[0:64, :])
                                nc.vector.tensor_copy(
                                    qropeTz[ho][64:128, tcol], pt[64:128, :])

                # duplicate k_rope^T rows for the padded rope matmuls
                nc.sync.dma_start(kropeT[64:128, :], kropeT[0:64, :])

                # ---- phase C: k_nope^T per head, v -> SBUF ------
                # prefetch phase-D proj weights during phase C compute
                for h in range(4):
                    nc.sync.dma_start(wproj_sb[h],
                                      wproj_d[h * P:(h + 1) * P, :])
                for h in range(4):
                    for n4 in range(4):
                        pk = psu.tile([P, 512], f32,
                                      tag=f"bk{(h * 4 + n4) % 2}",
                                      name="kn")
                        for kc in range(4):
                            nc.tensor.matmul(
                                pk, r(wdn[:, kc, h * 128:(h + 1) * 128]),
                                r(ckvT[kc][:, n4 * 512:(n4 + 1) * 512]),
                                start=(kc == 0), stop=(kc == 3))
                        nc.vector.tensor_copy(
                            knopeT[h][:, n4 * 512:(n4 + 1) * 512], pk)
                for tb in range(TB):
                    pv = psu.tile([P, 512], f32,
                                  tag=f"bk{2 + tb % 2}", name="v")
                    for kc in range(4):
                        nc.tensor.matmul(
                            pv, r(ckvT[kc][:, tb * P:(tb + 1) * P]),
                            r(wdv[:, kc, :]),
                            start=(kc == 0), stop=(kc == 3))
                    nc.vector.tensor_copy(vSB[:, tb, :], pv)

            pab_cm.__exit__(None, None, None)

            # ---- phase D: attention + proj -------------------------------
            with tc.tile_pool(name="mgp", bufs=8) as mgp, \
                 tc.tile_pool(name="sp", bufs=8) as sp, \
                 tc.tile_pool(name="stD", bufs=3) as stD, \
                 tc.tile_pool(name="attp", bufs=2) as attp:
                for j in range(NQ):
                    chunks = plan[j]
                    nchunks = len(chunks)
                    mg_tiles = {}
                    for (c, col0, sub) in chunks:
                        for s in sub:
                            if not isinstance(s, str):
                                mt = mgp.tile([P, P], f32, tag="mg",
                                              name="mg")
                                nc.sync.dma_start(mt, maskg_d[s[1]])
                                mg_tiles[s[1]] = mt
                    attT = {}
                    for g in range(4):  # heads, pipelined sequentially
                        h = g
                        qn = qnopeSB[:, h, j * 512:(j + 1) * 512]
                        ps_att = psu.tile([P, 512], f32,
                                          tag=f"bk{4 + g % 2}", name="att")
                        ps_l = psu.tile([P, 512], f32, tag="bk3", name="l")

                        def scores_mm(ci):
                            c, col0, sub = chunks[ci]
                            qsl = slice(512 * j + col0, 512 * (j + 1))
                            kcl = slice(128 * c, 128 * (c + 1))
                            ps_s = psu.tile([P, 512], f32,
                                            tag=f"bk{ci % 3}", name="s")
                            nc.tensor.matmul(
                                ps_s[:, col0:], r(knopeT[h][:, kcl]),
                                r(qn[:, col0:]),
                                start=True, stop=False)
                            nc.tensor.matmul(
                                ps_s[:, col0:], r(kropeT[:, kcl]),
                                r(qropeTz[h][:, qsl]),
                                start=False, stop=True)
                            return ps_s

                        def exp_mask(ci, ps_s):
                            c, col0, sub = chunks[ci]
                            sprime = sp.tile([P, 512], mdt, tag="sp",
                                             name="sp")
                            nc.scalar.activation(
                                sprime[:, col0:], ps_s[:, col0:],
                                AF.Exp, scale=SCALE)
                            for qs, s in enumerate(sub):
                                colA, colB = 128 * qs, 128 * (qs + 1)
                                if colA < col0 or s == "zero":
                                    continue
                                if s == "skip":
                                    nc.any.memset(
                                        zero_view(sprime[:, colA:colB]), 0.0)
                                else:
                                    mt = mg_tiles[s[1]]
                                    stt = stD.tile([P, P], f32, tag="stt",
                                                   name="stt")
                                    nc.vector.scalar_tensor_tensor(
                                        stt, ps_s[:, colA:colB], SCALE, mt,
                                        AL.mult, AL.add)
                                    nc.scalar.activation(
                                        sprime[:, colA:colB], stt, AF.Exp,
                                        scale=1.0)
                            return sprime

                        def l_mm(ci, sprime):
                            # row-sum via ones-matmul, interleaved with the
                            # chunk stream so consecutive same-bank
                            # accumulations never run back-to-back
                            c, col0, sub = chunks[ci]
                            nc.tensor.matmul(
                                ps_l[:, col0:], r(ones_m),
                                r(sprime[:, col0:]),
                                start=(ci == 0), stop=(ci == nchunks - 1))

                        def att_mm(ci, sprime):
                            c, col0, sub = chunks[ci]
                            nc.tensor.matmul(
                                ps_att[:, col0:],
                                r(vSB[:, c, h * 128:(h + 1) * 128]),
                                r(sprime[:, col0:]),
                                start=(ci == 0), stop=(ci == nchunks - 1))

                        pend = [scores_mm(0)]
                        for pi in range(1, min(3, nchunks)):
                            pend.append(scores_mm(pi))
                        for ci in range(nchunks):
                            sprime = exp_mask(ci, pend[ci])
                            if ci + 3 < nchunks:
                                pend.append(scores_mm(ci + 3))
                            att_mm(ci, sprime)
                            l_mm(ci, sprime)
                        lnl = stD.tile([P, 512], f32, tag="lr", name="lr")
                        nc.scalar.activation(lnl, ps_l, AF.Ln)
                        rec = stD.tile([P, 512], f32, tag="lr", name="lr")
                        nc.scalar.activation(rec, lnl, AF.Exp, scale=-1.0)
                        at = attp.tile([P, 512], mdt, tag=f"at{h}",
                                       name=f"at{h}")
                        nc.vector.tensor_tensor(at, ps_att, rec, AL.mult)
                        attT[h] = at
                    for qs in range(4):
                        for ct in range(4):
                            pso = psu.tile([P, 512], f32,
                                           tag=f"bk{6 + ct % 2}", name="o")
                            for h in range(4):
                                nc.tensor.matmul(
                                    pso,
                                    r(attT[h][:, qs * 128:(qs + 1) * 128]),
                                    r(wproj_sb[h][:,
                                                  ct * 512:(ct + 1) * 512]),
                                    start=(h == 0), stop=(h == 3))
                            ost = sp.tile([P, 512], f32, tag="ost",
                                          name="ost", bufs=3)
                            nc.vector.tensor_copy(ost, pso)
                            nc.sync.dma_start(
                                out_d[512 * j + 128 * qs:
                                      512 * j + 128 * (qs + 1),
                                      ct * 512:(ct + 1) * 512], ost)

            psu_cm.__exit__(None, None, None)

    orig_tables = bacc.get_activation_tables
    bacc.get_activation_tables = _act_tables_combined_only
    try:
        nc.compile()
    finally:
        bacc.get_activation_tables = orig_tables
    return nc


# ---------------------------------------------------------------- entry

def _ensure_axon_hook_shim():
    # bass_utils imports antenv.axon_hooks when tracing is requested via
    # env; provide a null hook module if the image lacks it so kernel()
    # never crashes on that path.
    try:
        import antenv.axon_hooks  # noqa: F401
    except Exception:
        import sys
        import types
        m = types.ModuleType("antenv.axon_hooks")
        _h = [None]
        m.set_axon_ntff_profile_hook = lambda h: _h.__setitem__(0, h)
        m.get_axon_ntff_profile_hook = lambda: _h[0]
        sys.modules["antenv.axon_hooks"] = m
        try:
            import antenv
            antenv.axon_hooks = m
        except Exception:
            pass


def kernel(x, freq_cis, mask, window, Wq, Wckv, kv_norm_w, Wdkv, Wproj,
           start_pos):
    global LAST_RESULTS
    _ensure_axon_hook_shim()
    from concourse.bass_utils import run_bass_kernel_spmd

    x = np.asarray(x, np.float32)
    freq_cis = np.asarray(freq_cis, np.float32)
    mask = np.asarray(mask, np.float32)
    Wq = np.asarray(Wq, np.float32)
    Wckv = np.asarray(Wckv, np.float32)
    kv_norm_w = np.asarray(kv_norm_w, np.float32)
    Wdkv = np.asarray(Wdkv, np.float32)
    Wproj = np.asarray(Wproj, np.float32)

    plan, maskg = _mask_plan(mask)
    key = (MM_DTYPE, _plan_key(plan))
    if key not in _prog_cache:
        _prog_cache[key] = _build(plan, maskg.shape[0])
    nc = _prog_cache[key]

    cosT = np.ascontiguousarray(freq_cis[:, :, 0])
    sinT = np.ascontiguousarray(freq_cis[:, :, 1])
    wckv_p = _pack_wckv(Wckv)

    in_maps = []
    for core in range(N_CORES):
        b, hg = core // 4, core % 4
        wdn, wdv = _pack_wdkv(Wdkv, kv_norm_w, hg)
        in_maps.append({
            "xT": np.ascontiguousarray(x[b].T),
            "wq": _pack_wq(Wq, hg),
            "wckv": wckv_p,
            "wdn": wdn,
            "wdv": wdv,
            "wproj": np.ascontiguousarray(Wproj[hg * 512:(hg + 1) * 512, :]),
            "cosT": cosT,
            "sinT": sinT,
            "maskg": maskg,
        })

    if MM_DTYPE == "bfloat16":
        import ml_dtypes
        mmdt = ml_dtypes.bfloat16
        for m in in_maps:
            for k in ("xT", "wq", "wckv", "wdn", "wdv", "wproj"):
                m[k] = m[k].astype(mmdt)

    res = run_bass_kernel_spmd(nc, in_maps, list(range(N_CORES)))
    LAST_RESULTS = res
    outs = [res.results[c]["out"] for c in range(N_CORES)]
    full = np.empty((B, T, C), np.float32)
    for b in range(B):
        full[b] = outs[4 * b] + outs[4 * b + 1] + outs[4 * b + 2] \
            + outs[4 * b + 3]
    return full


# revision 16
# speedup vs baseline: 1.1773x; 1.0232x over previous
"""MLA prefill kernel for 8 trn2 NeuronCores.

Sharding: core c handles batch b = c//4, head group hg = c%4 (4 of 16 heads).
Each core computes its 4 heads' attention + its partial proj output
[T, C]; the host sums the 4 partials per batch (unshard of the
head-contracted proj output) and stacks batches.

Device dataflow per core (matmuls in bf16, psum fp32):
  A: q_nope^T [4x128, T] -> SBUF-resident qnopeSB; q_rope [t,d] rotated
     -> SBUF-resident qrotSB
  B: ckv [t, 576] -> rmsnorm(lat), rotate k_rope, PE-transpose to
     ckv'^T [4x128, T], kropeT(dup) [128, T], qropeTz [4x(128, T)]
     zero-padded per head so phase D rope matmuls contract K=128
  C: k_nope^T per head [128, T], v [t, 4*128] -> SBUF-resident vSB
  D: per (head, q-block 512): S^T tiles [128k, 512q] (nope+rope matmuls),
     exp via ACT from PSUM (masked subblocks via DVE STT), row-sum l via
     DVE accumulation of sprime chunks + one fp32r ones-matmul,
     att^T accum via V-matmul; normalize by exp(-ln l); proj accumulates
     4 head-chunks -> direct PSUM->HBM DMA.
  One PSUM pool with per-bank tags spans all phases (no pool barriers).
"""

import numpy as np

B, T, C, H = 2, 2048, 2048, 16
NOPE, ROPE, VD, LORA = 128, 64, 128, 512
QK = NOPE + ROPE
EPS = 1e-6
SCALE = 1.0 / float(np.sqrt(QK))
P = 128
KC = C // P    # 16 contraction chunks over C
TB = T // P    # 16 token sub-blocks
NQ = T // 512  # 4 q-blocks
HPC = 4        # heads per core
N_CORES = 8

_prog_cache = {}
LAST_RESULTS = None  # BassKernelResults of the most recent run (for test.py)
MM_DTYPE = "bfloat16"  # "bfloat16" or "float32r" for all matmul operands


# ---------------------------------------------------------------- host prep

def _mask_plan(mask):
    """Classify mask into per-(q-block, k-chunk) plans.

    plan[j] = list of (c, col0, subops); subops[qs] in
    {"skip", "zero", ("g", gidx)} for columns [128*qs, 128*qs+128) of the
    S^T tile. col0 = 128 * (# leading skip sub-blocks), forced to 0 for
    the first chunk of each j. Chunks with all sub-blocks skip are
    omitted (their softmax contribution is exactly 0 in fp32).
    """
    plan = []
    gblocks = []
    for j in range(NQ):
        chunks = []
        for c in range(TB):
            sub = []
            nskip_lead = 0
            leading = True
            any_alive = False
            for qs in range(4):
                blk = mask[512 * j + 128 * qs: 512 * j + 128 * qs + 128,
                           128 * c: 128 * c + 128]
                if np.all(blk <= -88.0):
                    sub.append("skip")
                    if leading:
                        nskip_lead += 1
                elif np.all(blk == 0.0):
                    sub.append("zero")
                    leading = False
                    any_alive = True
                else:
                    gidx = len(gblocks)
                    gblocks.append(np.ascontiguousarray(blk.T))
                    sub.append(("g", gidx))
                    leading = False
                    any_alive = True
            if not any_alive:
                continue
            col0 = 128 * nskip_lead
            if not chunks:
                col0 = 0  # first chunk must initialize full psum width
            chunks.append((c, col0, sub))
        assert chunks, f"q-block {j}: all keys masked (unsupported)"
        plan.append(chunks)
    if gblocks:
        garr = np.stack(gblocks).astype(np.float32)
    else:
        garr = np.zeros((1, 128, 128), np.float32)
    return plan, garr


def _plan_key(plan):
    return tuple(
        tuple((c, col0, tuple(s if isinstance(s, str) else ("g",) for s in sub))
              for (c, col0, sub) in chunks)
        for chunks in plan
    )


def _pack_wq(Wq, hg):
    """[C, 768]: 4 heads' nope cols, then 2 rope 'pair' chunks laid out
    [h_even(32) h_odd(32) h'_even(32) h'_odd(32)]."""
    heads = [4 * hg + i for i in range(HPC)]
    cols = [Wq[:, h * QK: h * QK + NOPE] for h in heads]
    for h in heads:
        rope = Wq[:, h * QK + NOPE: h * QK + QK]
        cols.append(rope[:, 0::2])
        cols.append(rope[:, 1::2])
    return np.ascontiguousarray(np.concatenate(cols, axis=1))


def _pack_wckv(Wckv):
    """[C, 576]: lat 512 | rope_even 32 | rope_odd 32."""
    lat = Wckv[:, :LORA]
    rope = Wckv[:, LORA:]
    return np.ascontiguousarray(
        np.concatenate([lat, rope[:, 0::2], rope[:, 1::2]], axis=1))


def _pack_wdkv(Wdkv, kv_norm_w, hg):
    """(wdn [LORA, 512], wdv [LORA, 512]) with kv_norm_w folded in."""
    Wd = Wdkv * kv_norm_w[:, None]
    heads = [4 * hg + i for i in range(HPC)]
    n_cols = [Wd[:, h * (NOPE + VD): h * (NOPE + VD) + NOPE] for h in heads]
    v_cols = [Wd[:, h * (NOPE + VD) + NOPE: (h + 1) * (NOPE + VD)]
              for h in heads]
    return (np.ascontiguousarray(np.concatenate(n_cols, axis=1)),
            np.ascontiguousarray(np.concatenate(v_cols, axis=1)))


# ---------------------------------------------------------------- program

def _act_tables_combined_only(arch):
    """Steer Bacc's ACT table chooser to the one set containing Exp+Ln+Copy
    so the kernel pays a single ~2.7us table load instead of thrashing
    between exp_and_others and natural_log on every softmax normalize."""
    from concourse.hw_specs import get_activation_tables
    mine = {"Exp", "Ln", "Copy", "Identity", "Square", "MemsetZero"}
    t = get_activation_tables(arch)
    out = {}
    for name, fns in t.items():
        if name == "natural_log_exp_and_others" or not any(
                f.name in mine for f in fns):
            out[name] = fns
        else:
            out[name] = set()
    return out


def _build(plan, n_generic):
    import concourse.mybir as mybir
    import concourse.tile as tile
    from concourse import bacc
    from concourse.masks import make_identity

    f32 = mybir.dt.float32
    f32r = mybir.dt.float32r
    mdt = getattr(mybir.dt, MM_DTYPE)
    AL = mybir.AluOpType
    AF = mybir.ActivationFunctionType

    def r(ap):
        return ap

    def zero_view(ap):
        return ap.bitcast(f32) if MM_DTYPE == "float32r" else ap

    nc = bacc.Bacc(None, target_bir_lowering=False)

    xT_d = nc.dram_tensor("xT", [C, T], mdt, kind="ExternalInput")
    wq_d = nc.dram_tensor("wq", [C, 768], mdt, kind="ExternalInput")
    wckv_d = nc.dram_tensor("wckv", [C, 576], mdt, kind="ExternalInput")
    wdn_d = nc.dram_tensor("wdn", [LORA, 512], mdt, kind="ExternalInput")
    wdv_d = nc.dram_tensor("wdv", [LORA, 512], mdt, kind="ExternalInput")
    wproj_d = nc.dram_tensor("wproj", [512, C], mdt, kind="ExternalInput")
    cos_d = nc.dram_tensor("cosT", [T, 32], f32, kind="ExternalInput")
    sin_d = nc.dram_tensor("sinT", [T, 32], f32, kind="ExternalInput")
    maskg_d = nc.dram_tensor("maskg", [max(1, n_generic), 128, 128], f32,
                             kind="ExternalInput")
    out_d = nc.dram_tensor("out", [T, C], f32, kind="ExternalOutput")

    with tile.TileContext(nc) as tc:
        with tc.tile_pool(name="const", bufs=1) as const, \
             tc.tile_pool(name="p1", bufs=1) as p1, \
             tc.tile_pool(name="pkn", bufs=1) as pkn, \
             tc.tile_pool(name="pqv", bufs=1) as pqv:
            ident_f = const.tile([P, P], f32, tag="ident_f", name="ident_f")
            make_identity(nc, ident_f)
            ident = const.tile([P, P], mdt, tag="ident", name="ident")
            nc.scalar.copy(ident, ident_f)
            ones_f = const.tile([P, P], f32, tag="ones_f", name="ones_f")
            nc.any.memset(ones_f, 1.0)
            ones_m = const.tile([P, P], mdt, tag="ones_m", name="ones_m")
            nc.scalar.copy(ones_m, ones_f)
            eps_t = const.tile([P, 1], f32, tag="eps", name="eps")
            nc.any.memset(eps_t, EPS)
            cos_all = const.tile([P, TB, 32], f32, tag="cos", name="cos")
            sin_all = const.tile([P, TB, 32], f32, tag="sin", name="sin")

            # zero-padded per-head q_rope^T: head h occupies rows
            # 64*(h%2) .. 64*(h%2)+64, the other 64 rows are zero so the
            # phase-D rope matmul can contract all 128 partitions against
            # the duplicated kropeT (K=64 matmuls stream ~2x slower).
            qropeTz = [p1.tile([P, T], mdt, tag=f"qrTz{h}", name=f"qrTz{h}")
                       for h in range(4)]
            kropeT = p1.tile([P, T], mdt, tag="krT", name="krT")
            knopeT = [pkn.tile([P, T], mdt, tag=f"knT{h}", name=f"knT{h}")
                      for h in range(4)]
            # SBUF-resident intermediates (no DRAM scratch round-trips)
            qnopeSB = pqv.tile([P, 4, T], mdt, tag="qnSB", name="qnSB")
            vSB = pqv.tile([P, TB, 512], mdt, tag="vSB", name="vSB")
            wproj_sb = [pqv.tile([P, C], mdt, tag=f"wp{h}", name=f"wp{h}")
                        for h in range(4)]
            xT_r = xT_d[:].rearrange("(kc p) t -> p kc t", p=P)

            # one PSUM pool for the whole kernel: per-bank tags make bank
            # reuse a per-slot WAR dep; no pool-boundary barriers between
            # phases
            psu_cm = tc.tile_pool(name="psu", bufs=1, space="PSUM")
            psu = psu_cm.__enter__()

            # pool for tensors that live through phases A+B only
            pab_cm = tc.tile_pool(name="pab", bufs=1)
            pab = pab_cm.__enter__()
            qrotSB = pab.tile([P, TB, 256], mdt, tag="qrotSB", name="qrotSB")
            wckv_sb = pab.tile([P, KC, 576], mdt, tag="wckv", name="wckv")
            wckv_r = wckv_d[:].rearrange("(kc p) m -> p kc m", p=P)

            # ---- phase A: q_nope^T and rotated q_rope -> SBUF ----
            with tc.tile_pool(name="phA", bufs=1) as pA, \
                 tc.tile_pool(name="xa", bufs=4) as xa_pool, \
                 tc.tile_pool(name="stA", bufs=3) as stA:
                wqn = pA.tile([P, KC, 512], mdt, tag="wqn", name="wqn")
                wqr = pA.tile([P, KC, 256], mdt, tag="wqr", name="wqr")
                wq_r = wq_d[:].rearrange("(kc p) m -> p kc m", p=P)
                nc.sync.dma_start(wqn[:, 0:4], wq_r[:, 0:4, 0:512])

                for n in range(4):
                    pn = [psu.tile([P, 512], f32, tag=f"bk{m}",
                                   name=f"qn{m}") for m in range(4)]
                    pr = [psu.tile([P, 2, 256], f32, tag=f"bk{4 + s}",
                                   name=f"qr{s}") for s in range(2)]
                    # two bulk DMAs fetch all 16 xT chunks for this t-slice;
                    # each psum group then runs 16 back-to-back matmuls
                    xah = []
                    for hf in range(2):
                        xa = xa_pool.tile([P, 8, 512], mdt, tag="xa",
                                          name="xa")
                        nc.sync.dma_start(
                            xa, xT_r[:, 8 * hf:8 * hf + 8,
                                     n * 512:(n + 1) * 512])
                        xah.append(xa)

                    if n == 0:
                        # remaining weights queue behind the first-matmul
                        # critical loads, ordered by first compute use
                        for qk in range(1, 4):
                            nc.sync.dma_start(
                                wqn[:, 4 * qk:4 * qk + 4],
                                wq_r[:, 4 * qk:4 * qk + 4, 0:512])
                        nc.sync.dma_start(wqr, wq_r[:, :, 512:768])
                        nc.sync.dma_start(
                            cos_all,
                            cos_d[:].rearrange("(tb p) i -> p tb i", p=P))
                        nc.sync.dma_start(
                            sin_all,
                            sin_d[:].rearrange("(tb p) i -> p tb i", p=P))
                        # prefetch phase-B weights while phase A computes
                        for wk in range(4):
                            nc.sync.dma_start(
                                wckv_sb[:, 4 * wk:4 * wk + 4],
                                wckv_r[:, 4 * wk:4 * wk + 4])

                    def xat(k):
                        return xah[k // 8][:, k % 8]

                    for m in range(4):
                        for k in range(KC):
                            nc.tensor.matmul(
                                pn[m], r(wqn[:, k, m * 128:(m + 1) * 128]),
                                r(xat(k)), start=(k == 0), stop=(k == KC - 1))
                    for s4 in range(4):
                        for k in range(KC):
                            nc.tensor.matmul(
                                pr[s4 // 2][:, s4 % 2],
                                r(xat(k)[:, s4 * 128:(s4 + 1) * 128]),
                                r(wqr[:, k, :]),
                                start=(k == 0), stop=(k == KC - 1))
                    for m in range(4):
                        nc.scalar.copy(
                            qnopeSB[:, m, n * 512:(n + 1) * 512], pn[m])
                    for s in range(2):
                        # stage the rope psum to SBUF with a fast ACT copy
                        # so the psum bank (and the phase-A pool) frees
                        # without waiting on the serial DVE rotation chain
                        stg = stA.tile([P, 2, 256], f32, tag="stg",
                                       name="stg")
                        nc.scalar.copy(stg, pr[s])
                        tb0 = n * 4 + 2 * s
                        cosv = cos_all[:, tb0:tb0 + 2][:, :, None, :] \
                            .to_broadcast((P, 2, 4, 32))
                        sinv = sin_all[:, tb0:tb0 + 2][:, :, None, :] \
                            .to_broadcast((P, 2, 4, 32))
                        prv = stg.rearrange("p t (g i) -> p t g i", i=64)
                        qe, qo = prv[..., 0:32], prv[..., 32:64]
                        ta = stA.tile([P, 2, 128], f32, tag="ta",
                                      name="ta").rearrange(
                            "p t (g i) -> p t g i", i=32)
                        tb_ = stA.tile([P, 2, 128], f32, tag="tb",
                                       name="tb").rearrange(
                            "p t (g i) -> p t g i", i=32)
                        tc2 = stA.tile([P, 2, 128], f32, tag="tc",
                                       name="tc").rearrange(
                            "p t (g i) -> p t g i", i=32)
                        td = stA.tile([P, 2, 128], f32, tag="td",
                                      name="td").rearrange(
                            "p t (g i) -> p t g i", i=32)
                        qvv = qrotSB[:, tb0:tb0 + 2, :].rearrange(
                            "p t (g i) -> p t g i", i=64)
                        nc.vector.tensor_tensor(ta, qe, cosv, AL.mult)
                        nc.vector.tensor_tensor(tb_, qo, sinv, AL.mult)
                        nc.vector.tensor_tensor(qvv[..., 0:32], ta, tb_,
                                                AL.subtract)
                        nc.vector.tensor_tensor(tc2, qo, cosv, AL.mult)
                        nc.vector.tensor_tensor(td, qe, sinv, AL.mult)
                        nc.vector.tensor_tensor(qvv[..., 32:64], tc2, td,
                                                AL.add)

            # ---- phase B: ckv -> rms/rope -> transposed tensors ----------
            with tc.tile_pool(name="pckvT", bufs=1) as pckvT:
                ckvT = [pckvT.tile([P, T], mdt, tag=f"ckvT{d}",
                                   name=f"ckvT{d}") for d in range(4)]
                # phase-C weights: issue their loads before phase B compute
                wdn = pckvT.tile([P, 4, 512], mdt, tag="wdn", name="wdn")
                wdv = pckvT.tile([P, 4, 512], mdt, tag="wdv", name="wdv")
                nc.sync.dma_start(
                    wdn, wdn_d[:].rearrange("(kc p) m -> p kc m", p=P))
                nc.sync.dma_start(
                    wdv, wdv_d[:].rearrange("(kc p) m -> p kc m", p=P))

                # zero the dead half of each per-head q_rope^T tile once
                for h in range(4):
                    dead = slice(64, 128) if h % 2 == 0 else slice(0, 64)
                    nc.any.memset(zero_view(qropeTz[h][dead, :]), 0.0)

                with tc.tile_pool(name="xb", bufs=3) as xb_pool, \
                     tc.tile_pool(name="stB", bufs=3) as stB, \
                     tc.tile_pool(name="smB", bufs=4) as smB:
                    tr_idx = [0]
                    for n in range(8):  # 256-token slices
                        pcs = [[psu.tile([P, 288], f32,
                                         tag=f"bk{2 * s_ + u}",
                                         name=f"ckv{u}")
                                for u in range(2)] for s_ in range(2)]
                        xbh = []
                        for hf in range(2):
                            xb = xb_pool.tile([P, 8, 256], mdt, tag="xb",
                                              name="xb")
                            nc.sync.dma_start(
                                xb, xT_r[:, 8 * hf:8 * hf + 8,
                                         n * 256:(n + 1) * 256])
                            xbh.append(xb)

                        def xbt(k):
                            return xbh[k // 8][:, k % 8]

                        for s in range(2):
                            for u in range(2):
                                wsl = (slice(0, 288), slice(288, 576))[u]
                                for k in range(KC):
                                    nc.tensor.matmul(
                                        pcs[s][u],
                                        r(xbt(k)[:, s * 128:(s + 1) * 128]),
                                        r(wckv_sb[:, k, wsl]),
                                        start=(k == 0), stop=(k == KC - 1))
                        for s in range(2):
                            tb = n * 2 + s
                            p0, p1_ = pcs[s]
                            sq = stB.tile([P, 288], f32, tag="sq", name="sq")
                            sq2 = stB.tile([P, 224], f32, tag="sq2",
                                           name="sq2")
                            ss0 = smB.tile([P, 1], f32, tag="ss0", name="ss0")
                            ss1 = smB.tile([P, 1], f32, tag="ss1", name="ss1")
                            nc.scalar.activation(sq, p0, AF.Square,
                                                 accum_out=ss0)
                            nc.scalar.activation(sq2, p1_[:, 0:224],
                                                 AF.Square, accum_out=ss1)
                            ssum = smB.tile([P, 1], f32, tag="ss", name="ss")
                            nc.vector.tensor_add(ssum, ss0, ss1)
                            lnv = smB.tile([P, 1], f32, tag="lnv", name="lnv")
                            nc.scalar.activation(lnv, ssum, AF.Ln,
                                                 bias=eps_t,
                                                 scale=1.0 / LORA)
                            rfac = smB.tile([P, 1], f32, tag="rfac",
                                            name="rfac")
                            nc.scalar.activation(rfac, lnv, AF.Exp,
                                                 scale=-0.5)
                            ckvn = stB.tile([P, 512], mdt, tag="ckvn",
                                            name="ckvn")
                            nc.scalar.mul(ckvn[:, 0:288], p0, rfac)
                            nc.scalar.mul(ckvn[:, 288:512], p1_[:, 0:224],
                                          rfac)
                            # k_rope rotation (raw latent, un-normalized)
                            ke, ko = p1_[:, 224:256], p1_[:, 256:288]
                            cosv, sinv = cos_all[:, tb], sin_all[:, tb]
                            ra = stB.tile([P, 32], f32, tag="ra", name="ra")
                            rb = stB.tile([P, 32], f32, tag="rb", name="rb")
                            rc = stB.tile([P, 32], f32, tag="rc", name="rc")
                            rd = stB.tile([P, 32], f32, tag="rd", name="rd")
                            krt = stB.tile([P, 64], mdt, tag="krt",
                                           name="krt")
                            nc.vector.tensor_tensor(ra, ke, cosv, AL.mult)
                            nc.vector.tensor_tensor(rb, ko, sinv, AL.mult)
                            nc.vector.tensor_tensor(krt[:, 0:32], ra, rb,
                                                    AL.subtract)
                            nc.vector.tensor_tensor(rc, ko, cosv, AL.mult)
                            nc.vector.tensor_tensor(rd, ke, sinv, AL.mult)
                            nc.vector.tensor_tensor(krt[:, 32:64], rc, rd,
                                                    AL.add)
                            # transposes -> persistent ^T tensors
                            tcol = slice(tb * 128, (tb + 1) * 128)
                            for dc in range(4):
                                pt = psu.tile([P, P], mdt,
                                              tag=f"bk{4 + tr_idx[0] % 4}",
                                              name="tr")
                                tr_idx[0] += 1
                                nc.tensor.transpose(
                                    pt, ckvn[:, dc * 128:(dc + 1) * 128],
                                    ident)
                                nc.vector.tensor_copy(ckvT[dc][:, tcol], pt)
                            pt = psu.tile([P, P], mdt,
                                          tag=f"bk{4 + tr_idx[0] % 4}",
                                          name="tr")
                            tr_idx[0] += 1
                            nc.tensor.transpose(pt[0:64, :], krt, ident)
                            nc.vector.tensor_copy(kropeT[0:64, tcol],
                                                  pt[0:64, :])
                            for pc in range(2):
                                pt = psu.tile([P, P], mdt,
                                              tag=f"bk{4 + tr_idx[0] % 4}",
                                              name="tr")
                                tr_idx[0] += 1
                                nc.tensor.transpose(
                                    pt,
                                    qrotSB[:, tb, pc * 128:(pc + 1) * 128],
                                    ident)
                                # split the head pair into zero-padded
                                # per-head tiles
                                he, ho = 2 * pc, 2 * pc + 1
                                nc.vector.tensor_copy(
                                    qropeTz[he][0:64, tcol], pt[0:64, :])
                                nc.vector.tensor_copy(
                                    qropeTz[ho][64:128, tcol], pt[64:128, :])

                # duplicate k_rope^T rows for the padded rope matmuls
                nc.sync.dma_start(kropeT[64:128, :], kropeT[0:64, :])

                # ---- phase C: k_nope^T per head, v -> SBUF ------
                # prefetch phase-D proj weights during phase C compute
                for h in range(4):
                    nc.sync.dma_start(wproj_sb[h],
                                      wproj_d[h * P:(h + 1) * P, :])
                for h in range(4):
                    for n4 in range(4):
                        pk = psu.tile([P, 512], f32,
                                      tag=f"bk{(h * 4 + n4) % 2}",
                                      name="kn")
                        for kc in range(4):
                            nc.tensor.matmul(
                                pk, r(wdn[:, kc, h * 128:(h + 1) * 128]),
                                r(ckvT[kc][:, n4 * 512:(n4 + 1) * 512]),
                                start=(kc == 0), stop=(kc == 3))
                        nc.vector.tensor_copy(
                            knopeT[h][:, n4 * 512:(n4 + 1) * 512], pk)
                for tb in range(TB):
                    pv = psu.tile([P, 512], f32,
                                  tag=f"bk{2 + tb % 2}", name="v")
                    for kc in range(4):
                        nc.tensor.matmul(
                            pv, r(ckvT[kc][:, tb * P:(tb + 1) * P]),
                            r(wdv[:, kc, :]),
                            start=(kc == 0), stop=(kc == 3))
                    nc.vector.tensor_copy(vSB[:, tb, :], pv)

            pab_cm.__exit__(None, None, None)

            # ---- phase D: attention + proj -------------------------------
            with tc.tile_pool(name="mgp", bufs=8) as mgp, \
                 tc.tile_pool(name="sp", bufs=8) as sp, \
                 tc.tile_pool(name="stD", bufs=3) as stD, \
                 tc.tile_pool(name="attp", bufs=2) as attp:

                def emit_proj(j, attT):
                    for qs in range(4):
                        for ct in range(4):
                            pso = psu.tile([P, 512], f32,
                                           tag=f"bk{6 + ct % 2}", name="o")
                            for h in range(4):
                                nc.tensor.matmul(
                                    pso,
                                    r(attT[h][:, qs * 128:(qs + 1) * 128]),
                                    r(wproj_sb[h][:,
                                                  ct * 512:(ct + 1) * 512]),
                                    start=(h == 0), stop=(h == 3))
                            ost = sp.tile([P, 512], f32, tag="ost",
                                          name="ost", bufs=3)
                            nc.vector.tensor_copy(ost, pso)
                            nc.sync.dma_start(
                                out_d[512 * j + 128 * qs:
                                      512 * j + 128 * (qs + 1),
                                      ct * 512:(ct + 1) * 512], ost)

                prev_proj = None
                for j in range(NQ):
                    chunks = plan[j]
                    nchunks = len(chunks)
                    mg_tiles = {}
                    for (c, col0, sub) in chunks:
                        for s in sub:
                            if not isinstance(s, str):
                                mt = mgp.tile([P, P], f32, tag="mg",
                                              name="mg")
                                nc.sync.dma_start(mt, maskg_d[s[1]])
                                mg_tiles[s[1]] = mt
                    attT = {}
                    for g in range(4):  # heads, pipelined sequentially
                        h = g
                        qn = qnopeSB[:, h, j * 512:(j + 1) * 512]
                        ps_att = psu.tile([P, 512], f32,
                                          tag=f"bk{4 + g % 2}", name="att")
                        ps_l = psu.tile([P, 512], f32, tag="bk3", name="l")

                        def scores_mm(ci):
                            c, col0, sub = chunks[ci]
                            qsl = slice(512 * j + col0, 512 * (j + 1))
                            kcl = slice(128 * c, 128 * (c + 1))
                            ps_s = psu.tile([P, 512], f32,
                                            tag=f"bk{ci % 3}", name="s")
                            nc.tensor.matmul(
                                ps_s[:, col0:], r(knopeT[h][:, kcl]),
                                r(qn[:, col0:]),
                                start=True, stop=False)
                            nc.tensor.matmul(
                                ps_s[:, col0:], r(kropeT[:, kcl]),
                                r(qropeTz[h][:, qsl]),
                                start=False, stop=True)
                            return ps_s

                        def exp_mask(ci, ps_s):
                            c, col0, sub = chunks[ci]
                            sprime = sp.tile([P, 512], mdt, tag="sp",
                                             name="sp")
                            nc.scalar.activation(
                                sprime[:, col0:], ps_s[:, col0:],
                                AF.Exp, scale=SCALE)
                            for qs, s in enumerate(sub):
                                colA, colB = 128 * qs, 128 * (qs + 1)
                                if colA < col0 or s == "zero":
                                    continue
                                if s == "skip":
                                    nc.any.memset(
                                        zero_view(sprime[:, colA:colB]), 0.0)
                                else:
                                    mt = mg_tiles[s[1]]
                                    stt = stD.tile([P, P], f32, tag="stt",
                                                   name="stt")
                                    nc.vector.scalar_tensor_tensor(
                                        stt, ps_s[:, colA:colB], SCALE, mt,
                                        AL.mult, AL.add)
                                    nc.scalar.activation(
                                        sprime[:, colA:colB], stt, AF.Exp,
                                        scale=1.0)
                            return sprime

                        def l_mm(ci, sprime):
                            # row-sum via ones-matmul, interleaved with the
                            # chunk stream so consecutive same-bank
                            # accumulations never run back-to-back
                            c, col0, sub = chunks[ci]
                            nc.tensor.matmul(
                                ps_l[:, col0:], r(ones_m),
                                r(sprime[:, col0:]),
                                start=(ci == 0), stop=(ci == nchunks - 1))

                        def att_mm(ci, sprime):
                            c, col0, sub = chunks[ci]
                            nc.tensor.matmul(
                                ps_att[:, col0:],
                                r(vSB[:, c, h * 128:(h + 1) * 128]),
                                r(sprime[:, col0:]),
                                start=(ci == 0), stop=(ci == nchunks - 1))

                        pend = [scores_mm(0)]
                        for pi in range(1, min(3, nchunks)):
                            pend.append(scores_mm(pi))
                        for ci in range(nchunks):
                            sprime = exp_mask(ci, pend[ci])
                            if ci + 3 < nchunks:
                                pend.append(scores_mm(ci + 3))
                            att_mm(ci, sprime)
                            l_mm(ci, sprime)
                        lnl = stD.tile([P, 512], f32, tag="lr", name="lr")
                        nc.scalar.activation(lnl, ps_l, AF.Ln)
                        rec = stD.tile([P, 512], f32, tag="lr", name="lr")
                        nc.scalar.activation(rec, lnl, AF.Exp, scale=-1.0)
                        at = attp.tile([P, 512], mdt, tag=f"at{h}",
                                       name=f"at{h}")
                        nc.vector.tensor_tensor(at, ps_att, rec, AL.mult)
                        attT[h] = at
                        if g == 0 and prev_proj is not None:
                            # defer the previous q-block's proj until this
                            # q-block's first head has scores in flight, so
                            # the trailing normalize chain (ln/exp/mult of
                            # head 3) hides under matmul work
                            emit_proj(*prev_proj)
                            prev_proj = None
                    prev_proj = (j, attT)
                emit_proj(*prev_proj)

            psu_cm.__exit__(None, None, None)

    orig_tables = bacc.get_activation_tables
    bacc.get_activation_tables = _act_tables_combined_only
    try:
        nc.compile()
    finally:
        bacc.get_activation_tables = orig_tables
    return nc


# ---------------------------------------------------------------- entry

def _ensure_axon_hook_shim():
    # bass_utils imports antenv.axon_hooks when tracing is requested via
    # env; provide a null hook module if the image lacks it so kernel()
    # never crashes on that path.
    try:
        import antenv.axon_hooks  # noqa: F401
    except Exception:
        import sys
        import types
        m = types.ModuleType("antenv.axon_hooks")
        _h = [None]
        m.set_axon_ntff_profile_hook = lambda h: _h.__setitem__(0, h)
        m.get_axon_ntff_profile_hook = lambda: _h[0]
        sys.modules["antenv.axon_hooks"] = m
        try:
            import antenv
            antenv.axon_hooks = m
        except Exception:
            pass


def kernel(x, freq_cis, mask, window, Wq, Wckv, kv_norm_w, Wdkv, Wproj,
           start_pos):
    global LAST_RESULTS
    _ensure_axon_hook_shim()
    from concourse.bass_utils import run_bass_kernel_spmd

    x = np.asarray(x, np.float32)
    freq_cis = np.asarray(freq_cis, np.float32)
    mask = np.asarray(mask, np.float32)
    Wq = np.asarray(Wq, np.float32)
    Wckv = np.asarray(Wckv, np.float32)
    kv_norm_w = np.asarray(kv_norm_w, np.float32)
    Wdkv = np.asarray(Wdkv, np.float32)
    Wproj = np.asarray(Wproj, np.float32)

    plan, maskg = _mask_plan(mask)
    key = (MM_DTYPE, _plan_key(plan))
    if key not in _prog_cache:
        _prog_cache[key] = _build(plan, maskg.shape[0])
    nc = _prog_cache[key]

    cosT = np.ascontiguousarray(freq_cis[:, :, 0])
    sinT = np.ascontiguousarray(freq_cis[:, :, 1])
    wckv_p = _pack_wckv(Wckv)

    in_maps = []
    for core in range(N_CORES):
        b, hg = core // 4, core % 4
        wdn, wdv = _pack_wdkv(Wdkv, kv_norm_w, hg)
        in_maps.append({
            "xT": np.ascontiguousarray(x[b].T),
            "wq": _pack_wq(Wq, hg),
            "wckv": wckv_p,
            "wdn": wdn,
            "wdv": wdv,
            "wproj": np.ascontiguousarray(Wproj[hg * 512:(hg + 1) * 512, :]),
            "cosT": cosT,
            "sinT": sinT,
            "maskg": maskg,
        })

    if MM_DTYPE == "bfloat16":
        import ml_dtypes
        mmdt = ml_dtypes.bfloat16
        for m in in_maps:
            for k in ("xT", "wq", "wckv", "wdn", "wdv", "wproj"):
                m[k] = m[k].astype(mmdt)

    res = run_bass_kernel_spmd(nc, in_maps, list(range(N_CORES)))
    LAST_RESULTS = res
    outs = [res.results[c]["out"] for c in range(N_CORES)]
    full = np.empty((B, T, C), np.float32)
    for b in range(B):
        full[b] = outs[4 * b] + outs[4 * b + 1] + outs[4 * b + 2] \
            + outs[4 * b + 3]
    return full


# revision 22
# speedup vs baseline: 1.2293x; 1.0442x over previous
"""MLA prefill kernel for 8 trn2 NeuronCores.

Sharding: core c handles batch b = c//4, head group hg = c%4 (4 of 16 heads).
Each core computes its 4 heads' attention + its partial proj output
[T, C]; the host sums the 4 partials per batch (unshard of the
head-contracted proj output) and stacks batches.

Device dataflow per core (matmuls in bf16, psum fp32):
  A: q_nope^T [4x128, T] -> SBUF-resident qnopeSB; q_rope [t,d] rotated
     -> SBUF-resident qrotSB
  B: ckv [t, 576] -> rmsnorm(lat), rotate k_rope, PE-transpose to
     ckv'^T [4x128, T], kropeT(dup) [128, T], qropeTz [4x(128, T)]
     zero-padded per head so phase D rope matmuls contract K=128
  C: k_nope^T per head [128, T], v [t, 4*128] -> SBUF-resident vSB
  D: per (head, q-block 512): S^T tiles [128k, 512q] (nope+rope matmuls),
     exp via ACT from PSUM (masked subblocks via DVE STT), row-sum l via
     DVE accumulation of sprime chunks + one fp32r ones-matmul,
     att^T accum via V-matmul; normalize by exp(-ln l); proj accumulates
     4 head-chunks -> direct PSUM->HBM DMA.
  One PSUM pool with per-bank tags spans all phases (no pool barriers).
"""

import numpy as np

B, T, C, H = 2, 2048, 2048, 16
NOPE, ROPE, VD, LORA = 128, 64, 128, 512
QK = NOPE + ROPE
EPS = 1e-6
SCALE = 1.0 / float(np.sqrt(QK))
P = 128
KC = C // P    # 16 contraction chunks over C
TB = T // P    # 16 token sub-blocks
NQ = T // 512  # 4 q-blocks
HPC = 4        # heads per core
N_CORES = 8

_prog_cache = {}
LAST_RESULTS = None  # BassKernelResults of the most recent run (for test.py)
MM_DTYPE = "bfloat16"  # "bfloat16" or "float32r" for all matmul operands


# ---------------------------------------------------------------- host prep

def _mask_plan(mask):
    """Classify mask into per-(q-block, k-chunk) plans.

    plan[j] = list of (c, col0, subops); subops[qs] in
    {"skip", "zero", ("g", gidx)} for columns [128*qs, 128*qs+128) of the
    S^T tile. col0 = 128 * (# leading skip sub-blocks), forced to 0 for
    the first chunk of each j. Chunks with all sub-blocks skip are
    omitted (their softmax contribution is exactly 0 in fp32).
    """
    plan = []
    gblocks = []
    for j in range(NQ):
        chunks = []
        for c in range(TB):
            sub = []
            nskip_lead = 0
            leading = True
            any_alive = False
            for qs in range(4):
                blk = mask[512 * j + 128 * qs: 512 * j + 128 * qs + 128,
                           128 * c: 128 * c + 128]
                if np.all(blk <= -88.0):
                    sub.append("skip")
                    if leading:
                        nskip_lead += 1
                elif np.all(blk == 0.0):
                    sub.append("zero")
                    leading = False
                    any_alive = True
                else:
                    gidx = len(gblocks)
                    gblocks.append(np.ascontiguousarray(blk.T))
                    sub.append(("g", gidx))
                    leading = False
                    any_alive = True
            if not any_alive:
                continue
            col0 = 128 * nskip_lead
            if not chunks:
                col0 = 0  # first chunk must initialize full psum width
            chunks.append((c, col0, sub))
        assert chunks, f"q-block {j}: all keys masked (unsupported)"
        plan.append(chunks)
    if gblocks:
        garr = np.stack(gblocks).astype(np.float32)
    else:
        garr = np.zeros((1, 128, 128), np.float32)
    return plan, garr


def _plan_key(plan):
    return tuple(
        tuple((c, col0, tuple(s if isinstance(s, str) else ("g",) for s in sub))
              for (c, col0, sub) in chunks)
        for chunks in plan
    )


def _pack_wq(Wq, hg):
    """[C, 768]: 4 heads' nope cols, then 2 rope 'pair' chunks laid out
    [h_even(32) h_odd(32) h'_even(32) h'_odd(32)]."""
    heads = [4 * hg + i for i in range(HPC)]
    cols = [Wq[:, h * QK: h * QK + NOPE] for h in heads]
    for h in heads:
        rope = Wq[:, h * QK + NOPE: h * QK + QK]
        cols.append(rope[:, 0::2])
        cols.append(rope[:, 1::2])
    return np.ascontiguousarray(np.concatenate(cols, axis=1))


def _pack_wckv(Wckv):
    """[C, 576]: lat 512 | rope_even 32 | rope_odd 32."""
    lat = Wckv[:, :LORA]
    rope = Wckv[:, LORA:]
    return np.ascontiguousarray(
        np.concatenate([lat, rope[:, 0::2], rope[:, 1::2]], axis=1))


def _pack_wdkv(Wdkv, kv_norm_w, hg):
    """(wdn [LORA, 512], wdv [LORA, 512]) with kv_norm_w folded in."""
    Wd = Wdkv * kv_norm_w[:, None]
    heads = [4 * hg + i for i in range(HPC)]
    n_cols = [Wd[:, h * (NOPE + VD): h * (NOPE + VD) + NOPE] for h in heads]
    v_cols = [Wd[:, h * (NOPE + VD) + NOPE: (h + 1) * (NOPE + VD)]
              for h in heads]
    return (np.ascontiguousarray(np.concatenate(n_cols, axis=1)),
            np.ascontiguousarray(np.concatenate(v_cols, axis=1)))


# ---------------------------------------------------------------- program

def _act_tables_combined_only(arch):
    """Steer Bacc's ACT table chooser to the one set containing Exp+Ln+Copy
    so the kernel pays a single ~2.7us table load instead of thrashing
    between exp_and_others and natural_log on every softmax normalize."""
    from concourse.hw_specs import get_activation_tables
    mine = {"Exp", "Ln", "Copy", "Identity", "Square", "MemsetZero"}
    t = get_activation_tables(arch)
    out = {}
    for name, fns in t.items():
        if name == "natural_log_exp_and_others" or not any(
                f.name in mine for f in fns):
            out[name] = fns
        else:
            out[name] = set()
    return out


def _build(plan, n_generic):
    import concourse.mybir as mybir
    import concourse.tile as tile
    from concourse import bacc
    from concourse.masks import make_identity

    f32 = mybir.dt.float32
    f32r = mybir.dt.float32r
    mdt = getattr(mybir.dt, MM_DTYPE)
    AL = mybir.AluOpType
    AF = mybir.ActivationFunctionType

    def r(ap):
        return ap

    def zero_view(ap):
        return ap.bitcast(f32) if MM_DTYPE == "float32r" else ap

    nc = bacc.Bacc(None, target_bir_lowering=False)

    xT_d = nc.dram_tensor("xT", [C, T], mdt, kind="ExternalInput")
    wq_d = nc.dram_tensor("wq", [C, 768], mdt, kind="ExternalInput")
    wckv_d = nc.dram_tensor("wckv", [C, 576], mdt, kind="ExternalInput")
    wdn_d = nc.dram_tensor("wdn", [LORA, 512], mdt, kind="ExternalInput")
    wdv_d = nc.dram_tensor("wdv", [LORA, 512], mdt, kind="ExternalInput")
    wproj_d = nc.dram_tensor("wproj", [512, C], mdt, kind="ExternalInput")
    cos_d = nc.dram_tensor("cosT", [T, 32], f32, kind="ExternalInput")
    sin_d = nc.dram_tensor("sinT", [T, 32], f32, kind="ExternalInput")
    maskg_d = nc.dram_tensor("maskg", [max(1, n_generic), 128, 128], f32,
                             kind="ExternalInput")
    out_d = nc.dram_tensor("out", [T, C], f32, kind="ExternalOutput")

    with tile.TileContext(nc) as tc:
        with tc.tile_pool(name="const", bufs=1) as const, \
             tc.tile_pool(name="p1", bufs=1) as p1, \
             tc.tile_pool(name="pkn", bufs=1) as pkn, \
             tc.tile_pool(name="pqv", bufs=1) as pqv:
            ident_f = const.tile([P, P], f32, tag="ident_f", name="ident_f")
            make_identity(nc, ident_f)
            ident = const.tile([P, P], mdt, tag="ident", name="ident")
            nc.scalar.copy(ident, ident_f)
            ones_f = const.tile([P, P], f32, tag="ones_f", name="ones_f")
            nc.any.memset(ones_f, 1.0)
            ones_m = const.tile([P, P], mdt, tag="ones_m", name="ones_m")
            nc.scalar.copy(ones_m, ones_f)
            eps_t = const.tile([P, 1], f32, tag="eps", name="eps")
            nc.any.memset(eps_t, EPS)
            cos_all = const.tile([P, TB, 32], f32, tag="cos", name="cos")
            sin_all = const.tile([P, TB, 32], f32, tag="sin", name="sin")

            # zero-padded per-head q_rope^T: head h occupies rows
            # 64*(h%2) .. 64*(h%2)+64, the other 64 rows are zero so the
            # phase-D rope matmul can contract all 128 partitions against
            # the duplicated kropeT (K=64 matmuls stream ~2x slower).
            qropeTz = [p1.tile([P, T], mdt, tag=f"qrTz{h}", name=f"qrTz{h}")
                       for h in range(4)]
            kropeT = p1.tile([P, T], mdt, tag="krT", name="krT")
            knopeT = [pkn.tile([P, T], mdt, tag=f"knT{h}", name=f"knT{h}")
                      for h in range(4)]
            # SBUF-resident intermediates (no DRAM scratch round-trips)
            qnopeSB = pqv.tile([P, 4, T], mdt, tag="qnSB", name="qnSB")
            vSB = pqv.tile([P, TB, 512], mdt, tag="vSB", name="vSB")
            wproj_sb = [pqv.tile([P, C], mdt, tag=f"wp{h}", name=f"wp{h}")
                        for h in range(4)]
            xT_r = xT_d[:].rearrange("(kc p) t -> p kc t", p=P)

            # one PSUM pool for the whole kernel: per-bank tags make bank
            # reuse a per-slot WAR dep; no pool-boundary barriers between
            # phases
            psu_cm = tc.tile_pool(name="psu", bufs=1, space="PSUM")
            psu = psu_cm.__enter__()

            # pool for tensors that live through phases A+B only
            pab_cm = tc.tile_pool(name="pab", bufs=1)
            pab = pab_cm.__enter__()
            qrotSB = pab.tile([P, TB, 256], mdt, tag="qrotSB", name="qrotSB")
            wckv_sb = pab.tile([P, KC, 576], mdt, tag="wckv", name="wckv")
            wckv_r = wckv_d[:].rearrange("(kc p) m -> p kc m", p=P)
            # rope staging + xb pool live through phase B so nothing phase B
            # needs waits on phase-A pool teardown
            stab_cm = tc.tile_pool(name="stab", bufs=3)
            stab = stab_cm.__enter__()
            xb_cm = tc.tile_pool(name="xb", bufs=4)
            xb_pool = xb_cm.__enter__()

            def load_xb(n):
                xbh = []
                for hf in range(2):
                    xb = xb_pool.tile([P, 8, 256], mdt, tag="xb", name="xb")
                    nc.sync.dma_start(
                        xb, xT_r[:, 8 * hf:8 * hf + 8,
                                 n * 256:(n + 1) * 256])
                    xbh.append(xb)
                return xbh

            # ---- phase A: q_nope^T and rotated q_rope -> SBUF ----
            with tc.tile_pool(name="phA", bufs=1) as pA, \
                 tc.tile_pool(name="xa", bufs=4) as xa_pool:
                wqn = pA.tile([P, KC, 512], mdt, tag="wqn", name="wqn")
                wqr = pA.tile([P, KC, 256], mdt, tag="wqr", name="wqr")
                wq_r = wq_d[:].rearrange("(kc p) m -> p kc m", p=P)
                nc.sync.dma_start(wqn[:, 0:4], wq_r[:, 0:4, 0:512])

                def load_xa(n):
                    xah = []
                    for hf in range(2):
                        xa = xa_pool.tile([P, 8, 512], mdt, tag="xa",
                                          name="xa")
                        nc.sync.dma_start(
                            xa, xT_r[:, 8 * hf:8 * hf + 8,
                                     n * 512:(n + 1) * 512])
                        xah.append(xa)
                    return xah

                xa_next = load_xa(0)
                xb0 = None
                for n in range(4):
                    pn = [psu.tile([P, 512], f32, tag=f"bk{m}",
                                   name=f"qn{m}") for m in range(4)]
                    pr = [psu.tile([P, 2, 256], f32, tag=f"bk{4 + s}",
                                   name=f"qr{s}") for s in range(2)]
                    xah = xa_next

                    if n == 0:
                        # remaining weights queue behind the first-matmul
                        # critical loads, ordered by first compute use
                        for qk in range(1, 4):
                            nc.sync.dma_start(
                                wqn[:, 4 * qk:4 * qk + 4],
                                wq_r[:, 4 * qk:4 * qk + 4, 0:512])
                        nc.sync.dma_start(wqr, wq_r[:, :, 512:768])
                        nc.sync.dma_start(
                            cos_all,
                            cos_d[:].rearrange("(tb p) i -> p tb i", p=P))
                        nc.sync.dma_start(
                            sin_all,
                            sin_d[:].rearrange("(tb p) i -> p tb i", p=P))
                    if n < 3:
                        xa_next = load_xa(n + 1)
                    # phase-B prefetches staggered so they never delay the
                    # next x slice on the DMA queues
                    if n == 1:
                        for wk in range(2):
                            nc.sync.dma_start(
                                wckv_sb[:, 4 * wk:4 * wk + 4],
                                wckv_r[:, 4 * wk:4 * wk + 4])
                    if n == 2:
                        for wk in range(2, 4):
                            nc.sync.dma_start(
                                wckv_sb[:, 4 * wk:4 * wk + 4],
                                wckv_r[:, 4 * wk:4 * wk + 4])
                        xb0 = load_xb(0)

                    def xat(k):
                        return xah[k // 8][:, k % 8]

                    for m in range(4):
                        for k in range(KC):
                            nc.tensor.matmul(
                                pn[m], r(wqn[:, k, m * 128:(m + 1) * 128]),
                                r(xat(k)), start=(k == 0), stop=(k == KC - 1))
                    for s4 in range(4):
                        for k in range(KC):
                            nc.tensor.matmul(
                                pr[s4 // 2][:, s4 % 2],
                                r(xat(k)[:, s4 * 128:(s4 + 1) * 128]),
                                r(wqr[:, k, :]),
                                start=(k == 0), stop=(k == KC - 1))
                    for m in range(4):
                        nc.scalar.copy(
                            qnopeSB[:, m, n * 512:(n + 1) * 512], pn[m])
                    for s in range(2):
                        # stage the rope psum to SBUF with a fast ACT copy
                        # so the psum bank (and the phase-A pool) frees
                        # without waiting on the serial DVE rotation chain
                        stg = stab.tile([P, 2, 256], f32, tag="stg",
                                        name="stg")
                        nc.scalar.copy(stg, pr[s])
                        tb0 = n * 4 + 2 * s
                        cosv = cos_all[:, tb0:tb0 + 2][:, :, None, :] \
                            .to_broadcast((P, 2, 4, 32))
                        sinv = sin_all[:, tb0:tb0 + 2][:, :, None, :] \
                            .to_broadcast((P, 2, 4, 32))
                        prv = stg.rearrange("p t (g i) -> p t g i", i=64)
                        qe, qo = prv[..., 0:32], prv[..., 32:64]
                        ta = stab.tile([P, 2, 128], f32, tag="ta",
                                       name="ta").rearrange(
                            "p t (g i) -> p t g i", i=32)
                        tb_ = stab.tile([P, 2, 128], f32, tag="tb",
                                        name="tb").rearrange(
                            "p t (g i) -> p t g i", i=32)
                        tc2 = stab.tile([P, 2, 128], f32, tag="tc",
                                        name="tc").rearrange(
                            "p t (g i) -> p t g i", i=32)
                        td = stab.tile([P, 2, 128], f32, tag="td",
                                       name="td").rearrange(
                            "p t (g i) -> p t g i", i=32)
                        qvv = qrotSB[:, tb0:tb0 + 2, :].rearrange(
                            "p t (g i) -> p t g i", i=64)
                        nc.vector.tensor_tensor(ta, qe, cosv, AL.mult)
                        nc.vector.tensor_tensor(tb_, qo, sinv, AL.mult)
                        nc.vector.tensor_tensor(qvv[..., 0:32], ta, tb_,
                                                AL.subtract)
                        nc.vector.tensor_tensor(tc2, qo, cosv, AL.mult)
                        nc.vector.tensor_tensor(td, qe, sinv, AL.mult)
                        nc.vector.tensor_tensor(qvv[..., 32:64], tc2, td,
                                                AL.add)

            # ---- phase B: ckv -> rms/rope -> transposed tensors ----------
            with tc.tile_pool(name="pckvT", bufs=1) as pckvT:
                ckvT = [pckvT.tile([P, T], mdt, tag=f"ckvT{d}",
                                   name=f"ckvT{d}") for d in range(4)]
                # phase-C weights: issue their loads before phase B compute
                wdn = pckvT.tile([P, 4, 512], mdt, tag="wdn", name="wdn")
                wdv = pckvT.tile([P, 4, 512], mdt, tag="wdv", name="wdv")
                nc.sync.dma_start(
                    wdn, wdn_d[:].rearrange("(kc p) m -> p kc m", p=P))
                nc.sync.dma_start(
                    wdv, wdv_d[:].rearrange("(kc p) m -> p kc m", p=P))

                # zero the dead half of each per-head q_rope^T tile once
                for h in range(4):
                    dead = slice(64, 128) if h % 2 == 0 else slice(0, 64)
                    nc.any.memset(zero_view(qropeTz[h][dead, :]), 0.0)

                with tc.tile_pool(name="stB", bufs=3) as stB, \
                     tc.tile_pool(name="smB", bufs=4) as smB:
                    tr_idx = [0]
                    xb_next = xb0
                    for n in range(8):  # 256-token slices
                        pcs = [[psu.tile([P, 288], f32,
                                         tag=f"bk{2 * s_ + u}",
                                         name=f"ckv{u}")
                                for u in range(2)] for s_ in range(2)]
                        xbh = xb_next
                        if n < 7:
                            xb_next = load_xb(n + 1)

                        def xbt(k):
                            return xbh[k // 8][:, k % 8]

                        for s in range(2):
                            for u in range(2):
                                wsl = (slice(0, 288), slice(288, 576))[u]
                                for k in range(KC):
                                    nc.tensor.matmul(
                                        pcs[s][u],
                                        r(xbt(k)[:, s * 128:(s + 1) * 128]),
                                        r(wckv_sb[:, k, wsl]),
                                        start=(k == 0), stop=(k == KC - 1))
                        for s in range(2):
                            tb = n * 2 + s
                            p0, p1_ = pcs[s]
                            sq = stB.tile([P, 288], f32, tag="sq", name="sq")
                            sq2 = stB.tile([P, 224], f32, tag="sq2",
                                           name="sq2")
                            ss0 = smB.tile([P, 1], f32, tag="ss0", name="ss0")
                            ss1 = smB.tile([P, 1], f32, tag="ss1", name="ss1")
                            nc.scalar.activation(sq, p0, AF.Square,
                                                 accum_out=ss0)
                            nc.scalar.activation(sq2, p1_[:, 0:224],
                                                 AF.Square, accum_out=ss1)
                            ssum = smB.tile([P, 1], f32, tag="ss", name="ss")
                            nc.vector.tensor_add(ssum, ss0, ss1)
                            lnv = smB.tile([P, 1], f32, tag="lnv", name="lnv")
                            nc.scalar.activation(lnv, ssum, AF.Ln,
                                                 bias=eps_t,
                                                 scale=1.0 / LORA)
                            rfac = smB.tile([P, 1], f32, tag="rfac",
                                            name="rfac")
                            nc.scalar.activation(rfac, lnv, AF.Exp,
                                                 scale=-0.5)
                            ckvn = stB.tile([P, 512], mdt, tag="ckvn",
                                            name="ckvn")
                            nc.scalar.mul(ckvn[:, 0:288], p0, rfac)
                            nc.scalar.mul(ckvn[:, 288:512], p1_[:, 0:224],
                                          rfac)
                            # k_rope rotation (raw latent, un-normalized)
                            ke, ko = p1_[:, 224:256], p1_[:, 256:288]
                            cosv, sinv = cos_all[:, tb], sin_all[:, tb]
                            ra = stB.tile([P, 32], f32, tag="ra", name="ra")
                            rb = stB.tile([P, 32], f32, tag="rb", name="rb")
                            rc = stB.tile([P, 32], f32, tag="rc", name="rc")
                            rd = stB.tile([P, 32], f32, tag="rd", name="rd")
                            krt = stB.tile([P, 64], mdt, tag="krt",
                                           name="krt")
                            nc.vector.tensor_tensor(ra, ke, cosv, AL.mult)
                            nc.vector.tensor_tensor(rb, ko, sinv, AL.mult)
                            nc.vector.tensor_tensor(krt[:, 0:32], ra, rb,
                                                    AL.subtract)
                            nc.vector.tensor_tensor(rc, ko, cosv, AL.mult)
                            nc.vector.tensor_tensor(rd, ke, sinv, AL.mult)
                            nc.vector.tensor_tensor(krt[:, 32:64], rc, rd,
                                                    AL.add)
                            # transposes -> persistent ^T tensors
                            tcol = slice(tb * 128, (tb + 1) * 128)
                            for dc in range(4):
                                pt = psu.tile([P, P], mdt,
                                              tag=f"bk{4 + tr_idx[0] % 4}",
                                              name="tr")
                                tr_idx[0] += 1
                                nc.tensor.transpose(
                                    pt, ckvn[:, dc * 128:(dc + 1) * 128],
                                    ident)
                                nc.vector.tensor_copy(ckvT[dc][:, tcol], pt)
                            pt = psu.tile([P, P], mdt,
                                          tag=f"bk{4 + tr_idx[0] % 4}",
                                          name="tr")
                            tr_idx[0] += 1
                            nc.tensor.transpose(pt[0:64, :], krt, ident)
                            nc.vector.tensor_copy(kropeT[0:64, tcol],
                                                  pt[0:64, :])
                            for pc in range(2):
                                pt = psu.tile([P, P], mdt,
                                              tag=f"bk{4 + tr_idx[0] % 4}",
                                              name="tr")
                                tr_idx[0] += 1
                                nc.tensor.transpose(
                                    pt,
                                    qrotSB[:, tb, pc * 128:(pc + 1) * 128],
                                    ident)
                                # split the head pair into zero-padded
                                # per-head tiles
                                he, ho = 2 * pc, 2 * pc + 1
                                nc.vector.tensor_copy(
                                    qropeTz[he][0:64, tcol], pt[0:64, :])
                                nc.vector.tensor_copy(
                                    qropeTz[ho][64:128, tcol], pt[64:128, :])

                # duplicate k_rope^T rows for the padded rope matmuls
                nc.sync.dma_start(kropeT[64:128, :], kropeT[0:64, :])

                # ---- phase C: k_nope^T per head, v -> SBUF ------
                # prefetch phase-D proj weights during phase C compute
                for h in range(4):
                    nc.sync.dma_start(wproj_sb[h],
                                      wproj_d[h * P:(h + 1) * P, :])
                for h in range(4):
                    for n4 in range(4):
                        pk = psu.tile([P, 512], f32,
                                      tag=f"bk{(h * 4 + n4) % 2}",
                                      name="kn")
                        for kc in range(4):
                            nc.tensor.matmul(
                                pk, r(wdn[:, kc, h * 128:(h + 1) * 128]),
                                r(ckvT[kc][:, n4 * 512:(n4 + 1) * 512]),
                                start=(kc == 0), stop=(kc == 3))
                        nc.vector.tensor_copy(
                            knopeT[h][:, n4 * 512:(n4 + 1) * 512], pk)
                for tb in range(TB):
                    pv = psu.tile([P, 512], f32,
                                  tag=f"bk{2 + tb % 2}", name="v")
                    for kc in range(4):
                        nc.tensor.matmul(
                            pv, r(ckvT[kc][:, tb * P:(tb + 1) * P]),
                            r(wdv[:, kc, :]),
                            start=(kc == 0), stop=(kc == 3))
                    nc.vector.tensor_copy(vSB[:, tb, :], pv)

            xb_cm.__exit__(None, None, None)
            stab_cm.__exit__(None, None, None)
            pab_cm.__exit__(None, None, None)

            # ---- phase D: attention + proj -------------------------------
            with tc.tile_pool(name="mgp", bufs=8) as mgp, \
                 tc.tile_pool(name="sp", bufs=8) as sp, \
                 tc.tile_pool(name="stD", bufs=3) as stD, \
                 tc.tile_pool(name="attp", bufs=2) as attp:

                def emit_proj(j, attT):
                    for qs in range(4):
                        for ct in range(4):
                            pso = psu.tile([P, 512], f32,
                                           tag=f"bk{6 + ct % 2}", name="o")
                            for h in range(4):
                                nc.tensor.matmul(
                                    pso,
                                    r(attT[h][:, qs * 128:(qs + 1) * 128]),
                                    r(wproj_sb[h][:,
                                                  ct * 512:(ct + 1) * 512]),
                                    start=(h == 0), stop=(h == 3))
                            ost = sp.tile([P, 512], f32, tag="ost",
                                          name="ost", bufs=3)
                            nc.vector.tensor_copy(ost, pso)
                            nc.sync.dma_start(
                                out_d[512 * j + 128 * qs:
                                      512 * j + 128 * (qs + 1),
                                      ct * 512:(ct + 1) * 512], ost)

                prev_proj = None
                for j in range(NQ):
                    chunks = plan[j]
                    nchunks = len(chunks)
                    mg_tiles = {}
                    for (c, col0, sub) in chunks:
                        for s in sub:
                            if not isinstance(s, str):
                                mt = mgp.tile([P, P], f32, tag="mg",
                                              name="mg")
                                nc.sync.dma_start(mt, maskg_d[s[1]])
                                mg_tiles[s[1]] = mt
                    attT = {}
                    for g in range(4):  # heads, pipelined sequentially
                        h = g
                        qn = qnopeSB[:, h, j * 512:(j + 1) * 512]
                        ps_att = psu.tile([P, 512], f32,
                                          tag=f"bk{4 + g % 2}", name="att")
                        ps_l = psu.tile([P, 512], f32, tag="bk3", name="l")

                        def scores_mm(ci):
                            c, col0, sub = chunks[ci]
                            qsl = slice(512 * j + col0, 512 * (j + 1))
                            kcl = slice(128 * c, 128 * (c + 1))
                            ps_s = psu.tile([P, 512], f32,
                                            tag=f"bk{ci % 3}", name="s")
                            nc.tensor.matmul(
                                ps_s[:, col0:], r(knopeT[h][:, kcl]),
                                r(qn[:, col0:]),
                                start=True, stop=False)
                            nc.tensor.matmul(
                                ps_s[:, col0:], r(kropeT[:, kcl]),
                                r(qropeTz[h][:, qsl]),
                                start=False, stop=True)
                            return ps_s

                        def exp_mask(ci, ps_s):
                            c, col0, sub = chunks[ci]
                            sprime = sp.tile([P, 512], mdt, tag="sp",
                                             name="sp")
                            nc.scalar.activation(
                                sprime[:, col0:], ps_s[:, col0:],
                                AF.Exp, scale=SCALE)
                            for qs, s in enumerate(sub):
                                colA, colB = 128 * qs, 128 * (qs + 1)
                                if colA < col0 or s == "zero":
                                    continue
                                if s == "skip":
                                    nc.any.memset(
                                        zero_view(sprime[:, colA:colB]), 0.0)
                                else:
                                    mt = mg_tiles[s[1]]
                                    stt = stD.tile([P, P], f32, tag="stt",
                                                   name="stt")
                                    nc.vector.scalar_tensor_tensor(
                                        stt, ps_s[:, colA:colB], SCALE, mt,
                                        AL.mult, AL.add)
                                    nc.scalar.activation(
                                        sprime[:, colA:colB], stt, AF.Exp,
                                        scale=1.0)
                            return sprime

                        def l_mm(ci, sprime):
                            # row-sum via ones-matmul, interleaved with the
                            # chunk stream so consecutive same-bank
                            # accumulations never run back-to-back
                            c, col0, sub = chunks[ci]
                            nc.tensor.matmul(
                                ps_l[:, col0:], r(ones_m),
                                r(sprime[:, col0:]),
                                start=(ci == 0), stop=(ci == nchunks - 1))

                        def att_mm(ci, sprime):
                            c, col0, sub = chunks[ci]
                            nc.tensor.matmul(
                                ps_att[:, col0:],
                                r(vSB[:, c, h * 128:(h + 1) * 128]),
                                r(sprime[:, col0:]),
                                start=(ci == 0), stop=(ci == nchunks - 1))

                        pend = [scores_mm(0)]
                        for pi in range(1, min(3, nchunks)):
                            pend.append(scores_mm(pi))
                        for ci in range(nchunks):
                            sprime = exp_mask(ci, pend[ci])
                            if ci + 3 < nchunks:
                                pend.append(scores_mm(ci + 3))
                            att_mm(ci, sprime)
                            l_mm(ci, sprime)
                        lnl = stD.tile([P, 512], f32, tag="lr", name="lr")
                        nc.scalar.activation(lnl, ps_l, AF.Ln)
                        rec = stD.tile([P, 512], f32, tag="lr", name="lr")
                        nc.scalar.activation(rec, lnl, AF.Exp, scale=-1.0)
                        at = attp.tile([P, 512], mdt, tag=f"at{h}",
                                       name=f"at{h}")
                        nc.vector.tensor_tensor(at, ps_att, rec, AL.mult)
                        attT[h] = at
                        if g == 0 and prev_proj is not None:
                            # defer the previous q-block's proj until this
                            # q-block's first head has scores in flight, so
                            # the trailing normalize chain (ln/exp/mult of
                            # head 3) hides under matmul work
                            emit_proj(*prev_proj)
                            prev_proj = None
                    prev_proj = (j, attT)
                emit_proj(*prev_proj)

            psu_cm.__exit__(None, None, None)

    orig_tables = bacc.get_activation_tables
    bacc.get_activation_tables = _act_tables_combined_only
    try:
        nc.compile()
    finally:
        bacc.get_activation_tables = orig_tables
    return nc


# ---------------------------------------------------------------- entry

def _ensure_axon_hook_shim():
    # bass_utils imports antenv.axon_hooks when tracing is requested via
    # env; provide a null hook module if the image lacks it so kernel()
    # never crashes on that path.
    try:
        import antenv.axon_hooks  # noqa: F401
    except Exception:
        import sys
        import types
        m = types.ModuleType("antenv.axon_hooks")
        _h = [None]
        m.set_axon_ntff_profile_hook = lambda h: _h.__setitem__(0, h)
        m.get_axon_ntff_profile_hook = lambda: _h[0]
        sys.modules["antenv.axon_hooks"] = m
        try:
            import antenv
            antenv.axon_hooks = m
        except Exception:
            pass


def kernel(x, freq_cis, mask, window, Wq, Wckv, kv_norm_w, Wdkv, Wproj,
           start_pos):
    global LAST_RESULTS
    _ensure_axon_hook_shim()
    from concourse.bass_utils import run_bass_kernel_spmd

    x = np.asarray(x, np.float32)
    freq_cis = np.asarray(freq_cis, np.float32)
    mask = np.asarray(mask, np.float32)
    Wq = np.asarray(Wq, np.float32)
    Wckv = np.asarray(Wckv, np.float32)
    kv_norm_w = np.asarray(kv_norm_w, np.float32)
    Wdkv = np.asarray(Wdkv, np.float32)
    Wproj = np.asarray(Wproj, np.float32)

    plan, maskg = _mask_plan(mask)
    key = (MM_DTYPE, _plan_key(plan))
    if key not in _prog_cache:
        _prog_cache[key] = _build(plan, maskg.shape[0])
    nc = _prog_cache[key]

    cosT = np.ascontiguousarray(freq_cis[:, :, 0])
    sinT = np.ascontiguousarray(freq_cis[:, :, 1])
    wckv_p = _pack_wckv(Wckv)

    in_maps = []
    for core in range(N_CORES):
        b, hg = core // 4, core % 4
        wdn, wdv = _pack_wdkv(Wdkv, kv_norm_w, hg)
        in_maps.append({
            "xT": np.ascontiguousarray(x[b].T),
            "wq": _pack_wq(Wq, hg),
            "wckv": wckv_p,
            "wdn": wdn,
            "wdv": wdv,
            "wproj": np.ascontiguousarray(Wproj[hg * 512:(hg + 1) * 512, :]),
            "cosT": cosT,
            "sinT": sinT,
            "maskg": maskg,
        })

    if MM_DTYPE == "bfloat16":
        import ml_dtypes
        mmdt = ml_dtypes.bfloat16
        for m in in_maps:
            for k in ("xT", "wq", "wckv", "wdn", "wdv", "wproj"):
                m[k] = m[k].astype(mmdt)

    res = run_bass_kernel_spmd(nc, in_maps, list(range(N_CORES)))
    LAST_RESULTS = res
    outs = [res.results[c]["out"] for c in range(N_CORES)]
    full = np.empty((B, T, C), np.float32)
    for b in range(B):
        full[b] = outs[4 * b] + outs[4 * b + 1] + outs[4 * b + 2] \
            + outs[4 * b + 3]
    return full
